# revision 16
# baseline (speedup 1.0000x reference)
"""Trainium2 Bass kernel for DGNRNetwork (2-layer TransformerConv GNN + MLPs).

Strategy (8 NeuronCores, graph/data parallel):
  - Nodes padded to N_PAD=50176 and sharded by contiguous range: core c owns
    nodes [c*6272, (c+1)*6272), i.e. 49 blocks of 128 dst nodes per core.
  - Edges partitioned by dst shard on host, laid out BLOCK-MAJOR: for each
    dst block, its lo-half slots then hi-half slots (each padded to whole
    128-edge tiles, uniform across cores -> one SPMD program).  Each block's
    slots are contiguous, so S / S_T / kv_ch slices are contiguous and one
    DMA per block suffices.
  - k||v rows fetched with one indirect DMA (dma_gather) per (block, half),
    round-robined over the 4 SWDGE queues so descriptor generation runs
    concurrently on the 4 GpSimd core pairs.
  - qi = S_T_tile @ Q_blk on TensorE (host-precomputed one-hot S_T).
  - Per-edge logits on Vector (qi straight from PSUM); exp on Scalar into
    rhs[:, :, 0:4]; the v*alpha product reads the exp'd logits broadcast
    (stride-0) so no materialized [128,T,128] attention tile.
  - Segment softmax denominator and weighted sum are ONE accumulated TensorE
    matmul chain with the one-hot scatter matrix S.  Padding edges have
    all-zero S rows so they drop out.
  - Conv loop is stage-batched per chunk (stage1 qi+logits for all blocks,
    stage2 exp+rhs, stage3 aggregate+finish) so the in-order engines overlap
    across blocks instead of ping-ponging inside one block's serial chain.
  - Small weights replicated; kv tables exchanged with AllGather between
    layers; tiny Q-head computed redundantly, combined with masked AllReduce.
"""

import sys

sys.path.insert(0, "/opt/trn_rl_repo")

import numpy as np
import ml_dtypes

import concourse.bacc as bacc
import concourse.bass as bass
import concourse.mybir as mybir
import concourse.tile as tile
from concourse import bass_utils, library_config

F32 = mybir.dt.float32
BF16 = mybir.dt.bfloat16
I16 = mybir.dt.int16

N_CORES = 8


class Cfg:
    def __init__(self, n_nodes=50000, nblk=49, b=64, edge_bf16=True,
                 chunk_tiles=40, qi_group=4):
        self.N = n_nodes
        self.NBLK = nblk                 # dst blocks per core
        self.SHARD = nblk * 128          # nodes per core
        self.N_PAD = 8 * self.SHARD
        self.HALF = self.N_PAD // 2      # kv table split (int16 gather idx)
        self.B = b                       # batch (selected nodes)
        self.F_IN = 128
        self.H = 32
        self.HD = 128
        self.HEADS = 4
        self.EPS = 1e-16
        self.SCALE = 1.0 / np.sqrt(32.0)
        self.edge_bf16 = edge_bf16
        self.EDT = BF16 if edge_bf16 else F32
        self.EDT_NP = ml_dtypes.bfloat16 if edge_bf16 else np.float32
        self.s_fp8 = edge_bf16
        self.SDT = mybir.dt.float8e4 if self.s_fp8 else self.EDT
        self.SDT_NP = ml_dtypes.float8_e4m3 if self.s_fp8 else self.EDT_NP
        self.CHUNK_TILES = chunk_tiles   # max 128-edge tiles per chunk
        self.QI_GROUP = qi_group         # tiles per qi PSUM group (1 bank)
        assert self.N <= self.N_PAD and self.HALF < 32768


# --------------------------------------------------------------------------
# host-side preprocessing
# --------------------------------------------------------------------------


def _wrap16(values, slots):
    """dma_gather idx layout: idx i lives at [i % 16, i // 16], replicated
    across the eight 16-partition groups."""
    arr = np.zeros((16, slots // 16), dtype=np.int16)
    arr[np.arange(len(values)) % 16, np.arange(len(values)) // 16] = values
    return np.tile(arr, (8, 1))


def _plan_chunks(cfg, t_lo, t_hi):
    """Group consecutive blocks into chunks of <= CHUNK_TILES tiles.

    Block-major layout: block b occupies tiles [base[b], base[b]+t_lo[b]+
    t_hi[b]) -- lo tiles then hi tiles, contiguous.  Returns chunks: list of
    dicts with blocks, tile0 (global tile index of chunk start), tiles.
    """
    base = np.zeros(cfg.NBLK + 1, np.int64)
    for b in range(cfg.NBLK):
        base[b + 1] = base[b] + t_lo[b] + t_hi[b]
    chunks = []
    b = 0
    while b < cfg.NBLK:
        blocks = []
        tl = 0
        while b < cfg.NBLK:
            need = t_lo[b] + t_hi[b]
            if blocks and tl + need > cfg.CHUNK_TILES:
                break
            blocks.append(b)
            tl += need
            b += 1
        chunks.append(dict(blocks=blocks, tile0=int(base[blocks[0]]),
                           tiles=tl))
    return chunks, base, int(base[cfg.NBLK])




def _balance_perm(cfg, edge_index):
    """Per-core node->block packing: best-fit-decreasing into blocks capped
    at 1024 in-edges per src half (8 tiles), overflow concentrated in the
    trailing blocks.  Cores keep their node ranges (src lo/hi halves
    invariant); only positions within each shard permute.  Returns
    pos_of_old: old node id -> new node id."""
    esrc = np.asarray(edge_index[0]).astype(np.int64)
    edst = np.asarray(edge_index[1]).astype(np.int64)
    hi = (esrc >= cfg.HALF).astype(np.int64)
    deg = np.zeros((cfg.N_PAD, 2), np.int64)
    np.add.at(deg, (edst, hi), 1)

    CAP = 1024
    K = 3
    pos_of_old = np.zeros(cfg.N_PAD, np.int64)
    nb = cfg.NBLK
    nreg = nb - K
    for c in range(N_CORES):
        ids = np.arange(c * cfg.SHARD, (c + 1) * cfg.SHARD)
        dl = deg[ids, 0]
        dh = deg[ids, 1]
        order = np.argsort(-(dl + dh), kind="stable")
        fl = np.zeros(nb, np.int64)
        fh = np.zeros(nb, np.int64)
        cnt = np.zeros(nb, np.int64)
        assign = np.full(len(ids), -1, np.int64)
        leftover = []
        for i in order:
            ok = ((cnt[:nreg] < 128) & (fl[:nreg] + dl[i] <= CAP)
                  & (fh[:nreg] + dh[i] <= CAP))
            if ok.any():
                cand = np.where(ok)[0]
                score = np.maximum(fl[cand] + dl[i], fh[cand] + dh[i])
                b = int(cand[np.argmax(score)])
            else:
                leftover.append(i)
                continue
            assign[i] = b
            fl[b] += dl[i]
            fh[b] += dh[i]
            cnt[b] += 1
        leftover.sort(key=lambda i: -(dl[i] + dh[i]))
        for i in leftover:
            room = np.where(cnt < 128)[0]
            cand = room[room >= nreg] if (room >= nreg).any() else room
            b = int(cand[np.argmin(np.maximum(fl[cand] + dl[i],
                                              fh[cand] + dh[i]))])
            assign[i] = b
            fl[b] += dl[i]
            fh[b] += dh[i]
            cnt[b] += 1
        binorder = np.lexsort((np.arange(len(ids)), assign))
        rank = np.zeros(len(ids), np.int64)
        pos = np.zeros(nb, np.int64)
        for i in binorder:
            b = assign[i]
            rank[i] = pos[b]
            pos[b] += 1
        pos_of_old[ids] = c * cfg.SHARD + assign * 128 + rank
    return pos_of_old

def _prep_edges(cfg, edge_index):
    src = np.ascontiguousarray(edge_index[0]).astype(np.int64)
    dst = np.ascontiguousarray(edge_index[1]).astype(np.int64)
    core = dst // cfg.SHARD
    blk = (dst % cfg.SHARD) // 128
    hi = (src >= cfg.HALF).astype(np.int64)

    cnt = np.zeros((N_CORES, cfg.NBLK, 2), np.int64)
    np.add.at(cnt, (core, blk, hi), 1)
    t_lo = np.maximum(1, (cnt[:, :, 0].max(0) + 127) // 128)  # [NBLK]
    t_hi = np.maximum(1, (cnt[:, :, 1].max(0) + 127) // 128)

    chunks, base, total_tiles = _plan_chunks(cfg, t_lo.tolist(), t_hi.tolist())
    slots = total_tiles * 128
    assert slots % 16 == 0

    # global slot base for each (blk, hi) group (block-major layout)
    grp_base = np.zeros((cfg.NBLK, 2), np.int64)
    for b in range(cfg.NBLK):
        grp_base[b, 0] = base[b] * 128
        grp_base[b, 1] = (base[b] + t_lo[b]) * 128

    order = np.lexsort((src, hi, blk, core))
    s_src, s_dst, s_core, s_blk, s_hi = (
        src[order], dst[order], core[order], blk[order], hi[order])

    per_core = []
    for c in range(N_CORES):
        m = s_core == c
        csrc, cdst, cblk, chi = s_src[m], s_dst[m], s_blk[m], s_hi[m]
        # composite key non-decreasing under the sort above
        key = cblk * 2 + chi
        kcounts = np.bincount(key, minlength=cfg.NBLK * 2)
        kstarts = np.zeros_like(kcounts)
        kstarts[1:] = np.cumsum(kcounts)[:-1]
        rank = np.arange(len(key)) - kstarts[key]
        slot = grp_base[cblk, chi] + rank

        kv_val = np.where(chi == 1, csrc - cfg.HALF, csrc)
        kv_idx = np.zeros(slots, np.int64)
        kv_idx[slot] = kv_val

        S = np.zeros((128, slots), cfg.SDT_NP)
        scol = (slot // 128) * 128 + (cdst % 128)
        S[slot % 128, scol] = 1.0
        ST = np.zeros((128, slots), cfg.SDT_NP)
        stcol = (slot // 128) * 128 + (slot % 128)
        ST[cdst % 128, stcol] = 1.0

        per_core.append(dict(kv_idx=_wrap16(kv_idx, slots), S=S, ST=ST))
    return per_core, t_lo.tolist(), t_hi.tolist(), chunks, base, slots


def _prep_inputs(cfg, inputs):
    x = np.asarray(inputs["x"], np.float32)
    idx = np.asarray(inputs["idx"]).astype(np.int64)
    f32 = lambda k: np.ascontiguousarray(np.asarray(inputs[k], np.float32))

    ei = np.asarray(inputs["edge_index"]).astype(np.int64)
    pos_of_old = _balance_perm(cfg, ei)
    ei = pos_of_old[ei]
    idx = pos_of_old[idx]

    xp = np.zeros((cfg.N_PAD, cfg.F_IN), np.float32)
    old_of_new = np.argsort(pos_of_old)
    sel = old_of_new[old_of_new < cfg.N]
    xp[pos_of_old[sel]] = x[sel]

    per_core_e, t_lo, t_hi, chunks, base, slots = _prep_edges(cfg, ei)

    wkv1b = np.ascontiguousarray(np.concatenate([
        np.concatenate([f32("c1_wk"), f32("c1_wv")], axis=1),
        np.concatenate([f32("c1_bk"), f32("c1_bv")])[None, :]], axis=0))
    wq1b = np.ascontiguousarray(
        np.concatenate([f32("c1_wq"), f32("c1_bq")[None, :]], axis=0))
    enc_w2b = np.ascontiguousarray(
        np.concatenate([f32("enc_w2"), f32("enc_b2")[None, :]], axis=0))
    wkv2 = np.ascontiguousarray(
        np.concatenate([f32("c2_wk"), f32("c2_wv")], axis=1))     # [128,256]
    bkv2 = np.ascontiguousarray(
        np.concatenate([f32("c2_bk"), f32("c2_bv")])[None, :])
    qw1 = f32("q_w1")                                              # [288,128]
    bpad = ((cfg.B + 127) // 128) * 128

    in_maps = []
    for c in range(N_CORES):
        shard = slice(c * cfg.SHARD, (c + 1) * cfg.SHARD)
        own = (idx // cfg.SHARD) == c
        idx_loc = np.where(own, idx - c * cfg.SHARD, 0)
        im = dict(
            xT=np.ascontiguousarray(
                xp[shard].T.astype(ml_dtypes.bfloat16)),   # [128, SHARD]
            enc_w1=f32("enc_w1").astype(ml_dtypes.bfloat16),
            enc_b1=f32("enc_b1").reshape(32, 1),
            enc_w2=f32("enc_w2").astype(ml_dtypes.bfloat16),
            enc_b2c=f32("enc_b2").reshape(32, 1),
            enc_w2b=enc_w2b.astype(ml_dtypes.bfloat16),
            wq1b=wq1b.astype(ml_dtypes.bfloat16),
            wkv1b=wkv1b.astype(ml_dtypes.bfloat16),
            wq2=f32("c2_wq"), bq2=np.ascontiguousarray(f32("c2_bq")[None, :]),
            wkv2=wkv2, bkv2=bkv2,
            qw1a=np.ascontiguousarray(qw1[0:32]),
            qw1b=np.ascontiguousarray(qw1[32:160]),
            qw1c=np.ascontiguousarray(qw1[160:288]),
            qb1=f32("q_b1").reshape(128, 1),
            qw2=f32("q_w2"),
            qb2=f32("q_b2").reshape(1, 2),
            ones128=np.ones((1, 128), np.float32),
            id128=np.eye(128, dtype=np.float32),
            kv_idx=per_core_e[c]["kv_idx"],
            S_all=per_core_e[c]["S"],
            ST_all=per_core_e[c]["ST"],
            idx_x=_wrap16(idx_loc, bpad),
            own_mask=own.astype(np.float32).reshape(cfg.B, 1),
        )
        in_maps.append(im)
    return in_maps, t_lo, t_hi, chunks, base, slots


# --------------------------------------------------------------------------
# device program
# --------------------------------------------------------------------------


def build_program(cfg, t_lo, t_hi, chunks, base, slots):
    nc = bacc.Bacc("TRN2", target_bir_lowering=False, debug=False,
                   num_devices=N_CORES, num_swdge_queues=4)
    EDT = cfg.EDT
    NB, SH = cfg.NBLK, cfg.SHARD
    RG = [list(range(N_CORES))]
    RELU = mybir.ActivationFunctionType.Relu
    COPY = mybir.ActivationFunctionType.Copy
    EXP = mybir.ActivationFunctionType.Exp

    def din(name, shape, dt=F32):
        return nc.dram_tensor(name, list(shape), dt, kind="ExternalInput").ap()

    xT = din("xT", [128, SH], BF16)
    enc_w1 = din("enc_w1", [128, 32], BF16); enc_b1 = din("enc_b1", [32, 1])
    enc_w2 = din("enc_w2", [32, 32], BF16); enc_b2c = din("enc_b2c", [32, 1])
    enc_w2b = din("enc_w2b", [33, 32], BF16)
    wq1b = din("wq1b", [33, 128], BF16); wkv1b = din("wkv1b", [33, 256], BF16)
    wq2 = din("wq2", [128, 128]); bq2 = din("bq2", [1, 128])
    wkv2 = din("wkv2", [128, 256]); bkv2 = din("bkv2", [1, 256])
    qw1a = din("qw1a", [32, 128]); qw1b = din("qw1b", [128, 128])
    qw1c = din("qw1c", [128, 128]); qb1 = din("qb1", [128, 1])
    qw2 = din("qw2", [128, 2]); qb2 = din("qb2", [1, 2])
    ones128 = din("ones128", [1, 128]); id128 = din("id128", [128, 128])
    kv_idx_d = din("kv_idx", [128, slots // 16], I16)
    S_d = din("S_all", [128, slots], cfg.SDT)
    ST_d = din("ST_all", [128, slots], cfg.SDT)
    bpad = ((cfg.B + 127) // 128) * 128
    idx_x_d = din("idx_x", [128, bpad // 16], I16)
    own_mask_d = din("own_mask", [cfg.B, 1])
    out_d = nc.dram_tensor("out", [cfg.B, 2], F32, kind="ExternalOutput").ap()

    QI = cfg.QI_GROUP

    with tile.TileContext(nc) as tc:
        with (
            tc.tile_pool(name="const", bufs=1) as cpool,
            tc.tile_pool(name="kvch", bufs=3) as kvpool,
            tc.tile_pool(name="stp", bufs=5) as stpool,
            tc.tile_pool(name="sp", bufs=5) as sSpool,
            tc.tile_pool(name="qb", bufs=5) as qpool,
            tc.tile_pool(name="lt", bufs=5) as ltpool,
            tc.tile_pool(name="rhsp", bufs=4) as rhspool,
            tc.tile_pool(name="work", bufs=4) as wpool,
            tc.tile_pool(name="enc", bufs=14) as encpool,
            tc.tile_pool(name="small", bufs=3) as spool,
            tc.tile_pool(name="psQ", bufs=3, space="PSUM") as psQ,
            tc.tile_pool(name="psS", bufs=2, space="PSUM") as psS,
            tc.tile_pool(name="psT", bufs=3, space="PSUM") as psT,
            tc.tile_pool(name="dram", bufs=1, space="DRAM") as dpool,
        ):
            nc.gpsimd.load_library(library_config.mlp)

            def ld(ap, shape, dt=F32, nm=None):
                t = cpool.tile(shape, dt, name=nm or ("ld_" + ap.tensor.name))
                nc.sync.dma_start(t[:], ap[:])
                return t

            w_enc1 = ld(enc_w1, [128, 32], BF16)
            b_enc1 = ld(enc_b1, [32, 1])
            w_enc2 = ld(enc_w2, [32, 32], BF16)
            b_enc2c = ld(enc_b2c, [32, 1])
            w_enc2b = ld(enc_w2b, [33, 32], BF16)
            w_q1b = ld(wq1b, [33, 128], BF16)
            w_kv1b = ld(wkv1b, [33, 256], BF16)
            w_q2 = ld(wq2, [128, 128]); b_q2 = ld(bq2, [1, 128])
            w_kv2 = ld(wkv2, [128, 256]); b_kv2 = ld(bkv2, [1, 256])
            w_qha = ld(qw1a, [32, 128]); w_qhb = ld(qw1b, [128, 128])
            w_qhc = ld(qw1c, [128, 128])
            b_qh = ld(qb1, [128, 1]); w_qh2 = ld(qw2, [128, 2])
            b_qh2 = ld(qb2, [1, 2])
            ones_s = ld(ones128, [1, 128]); id_s = ld(id128, [128, 128])
            nidx16 = (cfg.B + 15) // 16
            idxx_s = cpool.tile([128, nidx16], I16, name="idxx_s")
            nc.sync.dma_start(idxx_s[:], idx_x_d[:, 0:nidx16])
            mask_s = ld(own_mask_d, [cfg.B, 1])
            kvidx_s = cpool.tile([128, slots // 16], I16, name="kvidx_s")
            nc.sync.dma_start(kvidx_s[:], kv_idx_d[:])

            q1_tab = dpool.tile([SH, 128], EDT, name="q1_tab")
            q2_tab = dpool.tile([SH, 128], EDT, name="q2_tab")
            kv1_sh = dpool.tile([SH, 256], EDT, name="kv1_sh")
            kv2_sh = dpool.tile([SH, 256], EDT, name="kv2_sh")
            kv1_full = dpool.tile([cfg.N_PAD, 256], EDT, name="kv1_full",
                                  addr_space="Shared")
            kv2_full = dpool.tile([cfg.N_PAD, 256], EDT, name="kv2_full",
                                  addr_space="Shared")
            h0_rows = dpool.tile([SH, 64], F32, name="h0_rows")
            h1_rows = dpool.tile([SH, 128], F32, name="h1_rows")
            h2_rows = dpool.tile([SH, 128], F32, name="h2_rows")
            ar_in = dpool.tile([cfg.B, 2], F32, name="ar_in")
            ar_out = dpool.tile([cfg.B, 2], F32, name="ar_out",
                                addr_space="Shared")

            # ===== encoder (own shard): kv1 rows first, then AllGather;
            # q1/h0 row emission overlaps the AllGather latency =====
            ENC_G = 4
            kept = []
            for g0 in range(0, NB, ENC_G):
                nb = min(ENC_G, NB - g0)
                W = nb * 128
                gsl = slice(g0 * 128, g0 * 128 + W)
                xch = wpool.tile([128, ENC_G * 128], BF16, tag="xch",
                                 name="xch")
                nc.sync.dma_start(xch[:, 0:W], xT[:, gsl])
                ps1 = psQ.tile([32, ENC_G * 128], F32, tag="psQ",
                               name="ps_enc1")
                nc.tensor.matmul(ps1[:, 0:W], w_enc1[:], xch[:, 0:W],
                                 start=True, stop=True)
                h1p = encpool.tile([33, ENC_G * 128], BF16, tag="h1p",
                                 name="h1p")
                nc.vector.memset(h1p[32:33, :], 1.0)
                nc.scalar.activation(h1p[0:32, 0:W], ps1[:, 0:W], RELU,
                                     bias=b_enc1[:], scale=1.0)
                ps2 = psQ.tile([32, ENC_G * 128], F32, tag="psQ",
                               name="ps_enc2")
                nc.tensor.matmul(ps2[:, 0:W], w_enc2[:], h1p[0:32, 0:W],
                                 start=True, stop=True)
                h0b = encpool.tile([33, ENC_G * 128], BF16, tag="h0b",
                                 name="h0b")
                nc.vector.memset(h0b[32:33, :], 1.0)
                nc.scalar.activation(h0b[0:32, 0:W], ps2[:, 0:W], RELU,
                                     bias=b_enc2c[:], scale=1.0)
                kept.append((g0, nb, h1p, h0b))
                for j in range(nb):
                    b = g0 + j
                    bsl = slice(b * 128, (b + 1) * 128)
                    jsl = slice(j * 128, (j + 1) * 128)
                    psk = psT.tile([128, 256], F32, tag="psT", name="ps_kv")
                    nc.tensor.matmul(psk[:], h0b[:, jsl], w_kv1b[:],
                                     start=True, stop=True)
                    kvr = spool.tile([128, 256], EDT, tag="kvr", name="kvr")
                    nc.vector.tensor_copy(kvr[:], psk[:])
                    nc.sync.dma_start(kv1_sh[bsl, :], kvr[:])
            nc.gpsimd.collective_compute(
                "AllGather", mybir.AluOpType.bypass, replica_groups=RG,
                ins=[kv1_sh.opt()], outs=[kv1_full.opt()])
            # q1 table + h0 rows (overlaps the AllGather)
            for g0, nb, h1p, h0b in kept:
                for j in range(nb):
                    b = g0 + j
                    bsl = slice(b * 128, (b + 1) * 128)
                    jsl = slice(j * 128, (j + 1) * 128)
                    psr = psT.tile([128, 32], F32, tag="psT", name="ps_h0r")
                    nc.tensor.matmul(psr[:], h1p[:, jsl], w_enc2b[:],
                                     start=True, stop=True)
                    h0r = spool.tile([128, 64], F32, tag="h0r", name="h0r")
                    nc.scalar.activation(h0r[:, 0:32], psr[:], RELU)
                    nc.sync.dma_start(h0_rows[bsl, :], h0r[:])
                    psq = psS.tile([128, 128], F32, tag="psS", name="ps_q")
                    nc.tensor.matmul(psq[:], h0b[:, jsl], w_q1b[:],
                                     start=True, stop=True)
                    qr = spool.tile([128, 128], EDT, tag="qr", name="qr")
                    nc.scalar.activation(qr[:], psq[:], COPY)
                    nc.sync.dma_start(q1_tab[bsl, :], qr[:])

            # ================= conv layers =================
            PREP_AHEAD = 0

            def emit_conv(q_tab, kv_full, h_rows_out, emit_tab2, lname):
                qrr = [0]

                def next_q():
                    q = qrr[0] & 3
                    qrr[0] += 1
                    return q

                def emit_gathers(ch, kv_ch, prep):
                    tile0 = ch["tile0"]
                    for b in ch["blocks"]:
                        for h in (0, 1):
                            nt = (t_hi if h else t_lo)[b]
                            toff = int(base[b]) - tile0 + (t_lo[b] if h else 0)
                            c8 = (int(base[b]) + (t_lo[b] if h else 0)) * 8
                            kw = {}
                            if prep:
                                kw = dict(prepare_only=True,
                                          sem=nc.alloc_semaphore(
                                              f"pg_{lname}_{b}_{h}"))
                            nc.gpsimd.dma_gather(
                                kv_ch[:, toff: toff + nt, :],
                                kv_full[h * cfg.HALF: (h + 1) * cfg.HALF, :],
                                kvidx_s[:, c8: c8 + nt * 8], nt * 128,
                                nt * 128, 256, single_packet=False,
                                queue_num=next_q(), **kw)

                # descriptor pre-generation for the first chunks: desc-gen
                # runs while the kv AllGather is still in flight; the
                # triggers (which carry the kv_full data dep) fire the DMAs
                # the moment the table lands.
                prep_tiles = []
                for ch in chunks[:PREP_AHEAD]:
                    kv_ch = kvpool.tile([128, ch["tiles"], 256], EDT,
                                        tag="kv_ch", name="kv_ch")
                    emit_gathers(ch, kv_ch, True)
                    prep_tiles.append(kv_ch)
                if prep_tiles:
                    for q in range(4):
                        nc.gpsimd.trigger_dma(count=None, queue_num=q)

                for ci, ch in enumerate(chunks):
                    blocks = ch["blocks"]
                    TC = ch["tiles"]
                    tile0 = ch["tile0"]

                    if ci < PREP_AHEAD:
                        kv_ch = prep_tiles[ci]
                    else:
                        kv_ch = kvpool.tile([128, TC, 256], EDT, tag="kv_ch",
                                            name="kv_ch")
                        emit_gathers(ch, kv_ch, False)

                    st_ts, s_ts, q_bs, l_ts, rhs_ts = {}, {}, {}, {}, {}
                    for b in blocks:
                        T = t_lo[b] + t_hi[b]
                        g0 = int(base[b]) * 128
                        st_t = stpool.tile([128, T * 128], cfg.SDT,
                                           tag="ST_b", name="ST_b")
                        nc.sync.dma_start(st_t[:], ST_d[:, g0: g0 + T * 128])
                        st_ts[b] = st_t
                        s_t = sSpool.tile([128, T * 128], cfg.SDT,
                                          tag="S_b", name="S_b")
                        nc.sync.dma_start(s_t[:], S_d[:, g0: g0 + T * 128])
                        s_ts[b] = s_t
                        bsl = slice(b * 128, (b + 1) * 128)
                        q_blk = qpool.tile([128, 128], EDT, tag="q_blk",
                                           name="q_blk")
                        nc.sync.dma_start(q_blk[:], q_tab[bsl, :])
                        q_bs[b] = q_blk

                    # stage 1: qi matmuls + logits
                    for b in blocks:
                        T = t_lo[b] + t_hi[b]
                        toff = int(base[b]) - tile0
                        st_t = st_ts[b]
                        l_t = ltpool.tile([128, T * 4], F32, tag="l_t",
                                          name="l_t")
                        l_ts[b] = l_t
                        for gs in range(0, T, QI):
                            g = min(QI, T - gs)
                            qi_ps = psQ.tile([128, QI, 128], F32, tag="psQ",
                                             name="qi_ps")
                            for i in range(g):
                                c0 = (gs + i) * 128
                                nc.tensor.matmul(
                                    qi_ps[:, i, :],
                                    st_t[:, c0: c0 + 128], q_bs[b][:],
                                    start=True, stop=True)
                            prod = spool.tile([128, QI, 128], EDT,
                                              tag="prod", name="prod")
                            nc.vector.tensor_tensor(
                                prod[:, 0:g, :],
                                qi_ps[:, 0:g, :],
                                kv_ch[:, toff + gs: toff + gs + g, 0:128],
                                mybir.AluOpType.mult)
                            nc.vector.tensor_reduce(
                                l_t[:, gs * 4: (gs + g) * 4].rearrange(
                                    "p (t h) -> p t h", h=4),
                                prod[:, 0:g, :].rearrange(
                                    "p t (h j) -> p t h j", h=4, j=32),
                                mybir.AxisListType.X, mybir.AluOpType.add)

                    # stage 2: exp + v*alpha (the multiply runs on GpSimd
                    # during conv1, whose gathers leave it half idle; Vector
                    # is the busier engine there)
                    veng = nc.gpsimd if emit_tab2 else nc.vector
                    for b in blocks:
                        T = t_lo[b] + t_hi[b]
                        toff = int(base[b]) - tile0
                        rhs = rhspool.tile([128, T, 132], EDT, tag="rhs",
                                           name="rhs")
                        rhs_ts[b] = rhs
                        nc.scalar.activation(
                            rhs[:, :, 0:4],
                            l_ts[b][:].rearrange("p (t h) -> p t h", h=4),
                            EXP, scale=float(cfg.SCALE))
                        veng.tensor_tensor(
                            rhs[:, :, 4:132].rearrange(
                                "p t (h j) -> p t h j", h=4, j=32),
                            kv_ch[:, toff: toff + T, 128:256].rearrange(
                                "p t (h j) -> p t h j", h=4, j=32),
                            rhs[:, :, 0:4].unsqueeze(-1).broadcast_to(
                                [128, T, 4, 32]),
                            mybir.AluOpType.mult)

                    # stage 3: aggregate + normalize + output (+ tab2)
                    for b in blocks:
                        T = t_lo[b] + t_hi[b]
                        bsl = slice(b * 128, (b + 1) * 128)
                        s_t = s_ts[b]
                        rhs = rhs_ts[b]
                        sc_ps = psS.tile([128, 132], F32, tag="psS",
                                         name="sc_ps")
                        for t in range(T):
                            nc.tensor.matmul(
                                sc_ps[:], s_t[:, t * 128: (t + 1) * 128],
                                rhs[:, t, :], start=(t == 0),
                                stop=(t == T - 1))

                        den = spool.tile([128, 4], F32, tag="den", name="den")
                        nc.vector.tensor_scalar_add(den[:], sc_ps[:, 0:4],
                                                    float(cfg.EPS))
                        rec = spool.tile([128, 4], F32, tag="rec", name="rec")
                        nc.vector.reciprocal(rec[:], den[:])
                        h_tmp = spool.tile([128, 128], F32, tag="h_tmp",
                                           name="h_tmp")
                        nc.vector.tensor_tensor(
                            h_tmp[:].rearrange("p (h j) -> p h j", h=4, j=32),
                            sc_ps[:, 4:132].rearrange(
                                "p (h j) -> p h j", h=4, j=32),
                            rec[:].unsqueeze(-1).broadcast_to([128, 4, 32]),
                            mybir.AluOpType.mult)
                        h_blk = spool.tile([128, 128], F32, tag="h_blk",
                                           name="h_blk")
                        nc.scalar.activation(h_blk[:], h_tmp[:], RELU)
                        nc.sync.dma_start(h_rows_out[bsl, :], h_blk[:])
                        if emit_tab2:
                            tr_ps = psT.tile([128, 128], F32, tag="psT",
                                             name="tr_ps")
                            nc.tensor.transpose(tr_ps[:], h_blk[:], id_s[:])
                            h1tb = spool.tile([128, 128], F32, tag="h1tb",
                                              name="h1tb")
                            nc.scalar.activation(h1tb[:], tr_ps[:], COPY)
                            # conv2 q/kv table rows, inline during conv1
                            psq = psS.tile([128, 128], F32, tag="psS",
                                           name="ps_q2")
                            nc.tensor.matmul(psq[:], h1tb[:], w_q2[:],
                                             start=True, stop=False)
                            nc.tensor.matmul(psq[:], ones_s[:], b_q2[:],
                                             start=False, stop=True)
                            qr = spool.tile([128, 128], EDT, tag="qr",
                                            name="qr2")
                            nc.scalar.activation(qr[:], psq[:], COPY)
                            nc.sync.dma_start(q2_tab[bsl, :], qr[:])
                            psk = psT.tile([128, 256], F32, tag="psT",
                                           name="ps_kv2")
                            nc.tensor.matmul(psk[:], h1tb[:], w_kv2[:],
                                             start=True, stop=False)
                            nc.tensor.matmul(psk[:], ones_s[:], b_kv2[:],
                                             start=False, stop=True)
                            kvr = spool.tile([128, 256], EDT, tag="kvr",
                                             name="kvr2")
                            nc.vector.tensor_copy(kvr[:], psk[:])
                            nc.sync.dma_start(kv2_sh[bsl, :], kvr[:])

            emit_conv(q1_tab, kv1_full, h1_rows, True, 'c1')
            nc.gpsimd.collective_compute(
                "AllGather", mybir.AluOpType.bypass, replica_groups=RG,
                ins=[kv2_sh.opt()], outs=[kv2_full.opt()])

            # ================= Q head =================
            def gather_xT(tab, width):
                g = spool.tile([128, 1, width], F32, tag="gx", name="gx")
                nc.gpsimd.dma_gather(g[:], tab[:, :], idxx_s[:],
                                     cfg.B, cfg.B, width)
                tp = psT.tile([128, 128], F32, tag="psT", name="tp_x")
                nc.tensor.transpose(tp[0:width, 0: cfg.B], g[0: cfg.B, 0, :],
                                    id_s[0: cfg.B, 0: cfg.B])
                xt = spool.tile([128, cfg.B], F32, tag="xt", name="xt")
                nc.scalar.activation(xt[0:width, :], tp[0:width, 0: cfg.B],
                                     COPY)
                return xt

            # x1/x2 part (h0/h1 ready once conv1 finished; overlaps conv2)
            x1t = gather_xT(h0_rows, 64)
            x2t = gather_xT(h1_rows, 128)
            zh12_ps = psS.tile([128, cfg.B], F32, tag="psS", name="zh12_ps")
            nc.tensor.matmul(zh12_ps[:], w_qha[:], x1t[0:32, :],
                             start=True, stop=False)
            nc.tensor.matmul(zh12_ps[:], w_qhb[:], x2t[0:128, :],
                             start=False, stop=True)
            zh12 = spool.tile([128, cfg.B], F32, tag="zh12", name="zh12")
            nc.scalar.activation(zh12[:], zh12_ps[:], COPY)

            emit_conv(q2_tab, kv2_full, h2_rows, False, 'c2')

            x3t = gather_xT(h2_rows, 128)
            zh_ps = psQ.tile([128, cfg.B], F32, tag="psQ", name="zh_ps")
            nc.tensor.matmul(zh_ps[:], w_qhc[:], x3t[0:128, :],
                             start=True, stop=True)
            zh = spool.tile([128, cfg.B], F32, tag="zh", name="zh")
            nc.vector.tensor_tensor(zh[:], zh_ps[:], zh12[:],
                                    mybir.AluOpType.add)
            zhr = spool.tile([128, cfg.B], F32, tag="zhr", name="zhr")
            nc.scalar.activation(zhr[:], zh[:], RELU, bias=b_qh[:],
                                 scale=1.0)
            o_ps = psS.tile([cfg.B, 2], F32, tag="psS", name="o_ps")
            nc.tensor.matmul(o_ps[:], zhr[:], w_qh2[:], start=True,
                             stop=False)
            nc.tensor.matmul(o_ps[:], ones_s[:, 0: cfg.B], b_qh2[:],
                             start=False, stop=True)
            ob = spool.tile([cfg.B, 2], F32, tag="ob", name="ob")
            nc.vector.tensor_scalar_mul(ob[:], o_ps[:], mask_s[:])
            nc.sync.dma_start(ar_in[:, :], ob[:])
            nc.gpsimd.collective_compute(
                "AllReduce", mybir.AluOpType.add, replica_groups=RG,
                ins=[ar_in.opt()], outs=[ar_out.opt()])
            nc.sync.dma_start(out_d[:, :], ar_out[:, :])

    nc.compile()
    return nc


# --------------------------------------------------------------------------
# entry point
# --------------------------------------------------------------------------

_trace_flag = {"trace": False}
_last = {}


def _chunk_key(chunks):
    return tuple((tuple(ch["blocks"]), ch["tiles"], ch["tile0"])
                 for ch in chunks)


def _run(inputs, cfg=None):
    cfg = cfg or Cfg()
    in_maps, t_lo, t_hi, chunks, base, slots = _prep_inputs(cfg, inputs)
    key = (slots, tuple(t_lo), tuple(t_hi), _chunk_key(chunks), cfg.edge_bf16)
    if _last.get("key") != key:
        _last["nc"] = build_program(cfg, t_lo, t_hi, chunks, base, slots)
        _last["key"] = key
    nc = _last["nc"]
    res = bass_utils.run_bass_kernel_spmd(
        nc, in_maps, core_ids=list(range(N_CORES)),
        trace=_trace_flag["trace"])
    _last["res"] = res
    return res.results[0]["out"].astype(np.float32)


def kernel(**inputs):
    return _run(inputs)


# revision 20
# speedup vs baseline: 1.3289x; 1.3289x over previous
"""Trainium2 Bass kernel for DGNRNetwork (2-layer TransformerConv GNN + MLPs).

Strategy (8 NeuronCores, graph/data parallel):
  - Nodes padded to N_PAD=50176 and sharded by contiguous range: core c owns
    nodes [c*6272, (c+1)*6272), i.e. 49 blocks of 128 dst nodes per core.
  - Edges partitioned by dst shard on host, laid out BLOCK-MAJOR: for each
    dst block, its lo-half slots then hi-half slots (each padded to whole
    128-edge tiles, uniform across cores -> one SPMD program).  Each block's
    slots are contiguous, so S / S_T / kv_ch slices are contiguous and one
    DMA per block suffices.
  - k||v rows fetched with one indirect DMA (dma_gather) per (block, half),
    round-robined over the 4 SWDGE queues so descriptor generation runs
    concurrently on the 4 GpSimd core pairs.
  - qi = S_T_tile @ Q_blk on TensorE (host-precomputed one-hot S_T).
  - Per-edge logits on Vector (qi straight from PSUM); exp on Scalar into
    rhs[:, :, 0:4]; the v*alpha product reads the exp'd logits broadcast
    (stride-0) so no materialized [128,T,128] attention tile.
  - Segment softmax denominator and weighted sum are ONE accumulated TensorE
    matmul chain with the one-hot scatter matrix S.  Padding edges have
    all-zero S rows so they drop out.
  - Conv loop is stage-batched per chunk (stage1 qi+logits for all blocks,
    stage2 exp+rhs, stage3 aggregate+finish) so the in-order engines overlap
    across blocks instead of ping-ponging inside one block's serial chain.
  - Small weights replicated; kv tables exchanged with AllGather between
    layers; tiny Q-head computed redundantly, combined with masked AllReduce.
"""

import sys

sys.path.insert(0, "/opt/trn_rl_repo")

import numpy as np
import ml_dtypes

import concourse.bacc as bacc
import concourse.bass as bass
import concourse.mybir as mybir
import concourse.tile as tile
from concourse import bass_utils, library_config

F32 = mybir.dt.float32
BF16 = mybir.dt.bfloat16
I16 = mybir.dt.int16

N_CORES = 8


class Cfg:
    def __init__(self, n_nodes=50000, nblk=49, b=64, edge_bf16=True,
                 chunk_tiles=40, qi_group=4):
        self.N = n_nodes
        self.NBLK = nblk                 # dst blocks per core
        self.SHARD = nblk * 128          # nodes per core
        self.N_PAD = 8 * self.SHARD
        self.HALF = self.N_PAD // 2      # kv table split (int16 gather idx)
        self.B = b                       # batch (selected nodes)
        self.F_IN = 128
        self.H = 32
        self.HD = 128
        self.HEADS = 4
        self.EPS = 1e-16
        self.SCALE = 1.0 / np.sqrt(32.0)
        self.edge_bf16 = edge_bf16
        self.EDT = BF16 if edge_bf16 else F32
        self.EDT_NP = ml_dtypes.bfloat16 if edge_bf16 else np.float32
        self.s_fp8 = edge_bf16
        self.SDT = mybir.dt.float8e4 if self.s_fp8 else self.EDT
        self.SDT_NP = ml_dtypes.float8_e4m3 if self.s_fp8 else self.EDT_NP
        self.CHUNK_TILES = chunk_tiles   # max 128-edge tiles per chunk
        self.QI_GROUP = qi_group         # tiles per qi PSUM group (1 bank)
        assert self.N <= self.N_PAD and self.HALF < 32768


# --------------------------------------------------------------------------
# host-side preprocessing
# --------------------------------------------------------------------------


def _wrap16(values, slots):
    """dma_gather idx layout: idx i lives at [i % 16, i // 16], replicated
    across the eight 16-partition groups."""
    arr = np.zeros((16, slots // 16), dtype=np.int16)
    arr[np.arange(len(values)) % 16, np.arange(len(values)) // 16] = values
    return np.tile(arr, (8, 1))


def _plan_chunks(cfg, t_lo, t_hi):
    """Group consecutive blocks into chunks of <= CHUNK_TILES tiles.

    Block-major layout: block b occupies tiles [base[b], base[b]+t_lo[b]+
    t_hi[b]) -- lo tiles then hi tiles, contiguous.  Returns chunks: list of
    dicts with blocks, tile0 (global tile index of chunk start), tiles.
    """
    base = np.zeros(cfg.NBLK + 1, np.int64)
    for b in range(cfg.NBLK):
        base[b + 1] = base[b] + t_lo[b] + t_hi[b]
    chunks = []
    b = 0
    while b < cfg.NBLK:
        blocks = []
        tl = 0
        while b < cfg.NBLK:
            need = t_lo[b] + t_hi[b]
            if blocks and tl + need > cfg.CHUNK_TILES:
                break
            blocks.append(b)
            tl += need
            b += 1
        chunks.append(dict(blocks=blocks, tile0=int(base[blocks[0]]),
                           tiles=tl))
    return chunks, base, int(base[cfg.NBLK])




def _balance_perm(cfg, edge_index):
    """Per-core node->block packing: best-fit-decreasing into blocks capped
    at 1024 in-edges per src half (8 tiles), overflow concentrated in the
    trailing blocks.  Cores keep their node ranges (src lo/hi halves
    invariant); only positions within each shard permute.  Returns
    pos_of_old: old node id -> new node id."""
    esrc = np.asarray(edge_index[0]).astype(np.int64)
    edst = np.asarray(edge_index[1]).astype(np.int64)
    hi = (esrc >= cfg.HALF).astype(np.int64)
    deg = np.zeros((cfg.N_PAD, 2), np.int64)
    np.add.at(deg, (edst, hi), 1)

    CAP = 1024
    K = 3
    pos_of_old = np.zeros(cfg.N_PAD, np.int64)
    nb = cfg.NBLK
    nreg = nb - K
    for c in range(N_CORES):
        ids = np.arange(c * cfg.SHARD, (c + 1) * cfg.SHARD)
        dl = deg[ids, 0]
        dh = deg[ids, 1]
        order = np.argsort(-(dl + dh), kind="stable")
        fl = np.zeros(nb, np.int64)
        fh = np.zeros(nb, np.int64)
        cnt = np.zeros(nb, np.int64)
        assign = np.full(len(ids), -1, np.int64)
        leftover = []
        for i in order:
            ok = ((cnt[:nreg] < 128) & (fl[:nreg] + dl[i] <= CAP)
                  & (fh[:nreg] + dh[i] <= CAP))
            if ok.any():
                cand = np.where(ok)[0]
                score = np.maximum(fl[cand] + dl[i], fh[cand] + dh[i])
                b = int(cand[np.argmax(score)])
            else:
                leftover.append(i)
                continue
            assign[i] = b
            fl[b] += dl[i]
            fh[b] += dh[i]
            cnt[b] += 1
        leftover.sort(key=lambda i: -(dl[i] + dh[i]))
        for i in leftover:
            room = np.where(cnt < 128)[0]
            cand = room[room >= nreg] if (room >= nreg).any() else room
            b = int(cand[np.argmin(np.maximum(fl[cand] + dl[i],
                                              fh[cand] + dh[i]))])
            assign[i] = b
            fl[b] += dl[i]
            fh[b] += dh[i]
            cnt[b] += 1
        binorder = np.lexsort((np.arange(len(ids)), assign))
        rank = np.zeros(len(ids), np.int64)
        pos = np.zeros(nb, np.int64)
        for i in binorder:
            b = assign[i]
            rank[i] = pos[b]
            pos[b] += 1
        pos_of_old[ids] = c * cfg.SHARD + assign * 128 + rank
    return pos_of_old

PIECE = 3072  # own-shard rows in AllGather piece 0 (block-aligned)


def _map_row(cfg, p):
    """node position -> kv table row under the two-piece rank-major
    AllGather layout."""
    r = p // cfg.SHARD
    o = p % cfg.SHARD
    l1 = cfg.SHARD - PIECE
    return np.where(o < PIECE, r * PIECE + o,
                    N_CORES * PIECE + r * l1 + (o - PIECE))


def _prep_edges(cfg, edge_index):
    src = np.ascontiguousarray(edge_index[0]).astype(np.int64)
    dst = np.ascontiguousarray(edge_index[1]).astype(np.int64)
    core = dst // cfg.SHARD
    blk = (dst % cfg.SHARD) // 128
    srow = _map_row(cfg, src)
    hi = (srow >= N_CORES * PIECE).astype(np.int64)

    cnt = np.zeros((N_CORES, cfg.NBLK, 2), np.int64)
    np.add.at(cnt, (core, blk, hi), 1)
    t_lo = np.maximum(1, (cnt[:, :, 0].max(0) + 127) // 128)  # [NBLK]
    t_hi = np.maximum(1, (cnt[:, :, 1].max(0) + 127) // 128)

    chunks, base, total_tiles = _plan_chunks(cfg, t_lo.tolist(), t_hi.tolist())
    slots = total_tiles * 128
    assert slots % 16 == 0

    # global slot base for each (blk, hi) group (block-major layout)
    grp_base = np.zeros((cfg.NBLK, 2), np.int64)
    for b in range(cfg.NBLK):
        grp_base[b, 0] = base[b] * 128
        grp_base[b, 1] = (base[b] + t_lo[b]) * 128

    order = np.lexsort((srow, hi, blk, core))
    s_src, s_dst, s_core, s_blk, s_hi = (
        srow[order], dst[order], core[order], blk[order], hi[order])

    per_core = []
    for c in range(N_CORES):
        m = s_core == c
        csrc, cdst, cblk, chi = s_src[m], s_dst[m], s_blk[m], s_hi[m]
        # composite key non-decreasing under the sort above
        key = cblk * 2 + chi
        kcounts = np.bincount(key, minlength=cfg.NBLK * 2)
        kstarts = np.zeros_like(kcounts)
        kstarts[1:] = np.cumsum(kcounts)[:-1]
        rank = np.arange(len(key)) - kstarts[key]
        slot = grp_base[cblk, chi] + rank

        kv_val = np.where(chi == 1, csrc - N_CORES * PIECE, csrc)
        kv_idx = np.zeros(slots, np.int64)
        kv_idx[slot] = kv_val

        S = np.zeros((128, slots), cfg.SDT_NP)
        scol = (slot // 128) * 128 + (cdst % 128)
        S[slot % 128, scol] = 1.0
        ST = np.zeros((128, slots), cfg.SDT_NP)
        stcol = (slot // 128) * 128 + (slot % 128)
        ST[cdst % 128, stcol] = 1.0

        per_core.append(dict(kv_idx=_wrap16(kv_idx, slots), S=S, ST=ST))
    return per_core, t_lo.tolist(), t_hi.tolist(), chunks, base, slots


def _prep_inputs(cfg, inputs):
    x = np.asarray(inputs["x"], np.float32)
    idx = np.asarray(inputs["idx"]).astype(np.int64)
    f32 = lambda k: np.ascontiguousarray(np.asarray(inputs[k], np.float32))

    ei = np.asarray(inputs["edge_index"]).astype(np.int64)
    pos_of_old = _balance_perm(cfg, ei)
    ei = pos_of_old[ei]
    idx = pos_of_old[idx]

    xp = np.zeros((cfg.N_PAD, cfg.F_IN), np.float32)
    old_of_new = np.argsort(pos_of_old)
    sel = old_of_new[old_of_new < cfg.N]
    xp[pos_of_old[sel]] = x[sel]

    per_core_e, t_lo, t_hi, chunks, base, slots = _prep_edges(cfg, ei)

    wkv1b = np.ascontiguousarray(np.concatenate([
        np.concatenate([f32("c1_wk"), f32("c1_wv")], axis=1),
        np.concatenate([f32("c1_bk"), f32("c1_bv")])[None, :]], axis=0))
    wq1b = np.ascontiguousarray(
        np.concatenate([f32("c1_wq"), f32("c1_bq")[None, :]], axis=0))
    enc_w2b = np.ascontiguousarray(
        np.concatenate([f32("enc_w2"), f32("enc_b2")[None, :]], axis=0))
    wkv2 = np.ascontiguousarray(
        np.concatenate([f32("c2_wk"), f32("c2_wv")], axis=1))     # [128,256]
    bkv2 = np.ascontiguousarray(
        np.concatenate([f32("c2_bk"), f32("c2_bv")])[None, :])
    qw1 = f32("q_w1")                                              # [288,128]
    bpad = ((cfg.B + 127) // 128) * 128

    in_maps = []
    for c in range(N_CORES):
        shard = slice(c * cfg.SHARD, (c + 1) * cfg.SHARD)
        own = (idx // cfg.SHARD) == c
        idx_loc = np.where(own, idx - c * cfg.SHARD, 0)
        im = dict(
            xT=np.ascontiguousarray(
                xp[shard].T.astype(ml_dtypes.bfloat16)),   # [128, SHARD]
            enc_w1=f32("enc_w1").astype(ml_dtypes.bfloat16),
            enc_b1=f32("enc_b1").reshape(32, 1),
            enc_w2=f32("enc_w2").astype(ml_dtypes.bfloat16),
            enc_b2c=f32("enc_b2").reshape(32, 1),
            enc_w2b=enc_w2b.astype(ml_dtypes.bfloat16),
            wq1b=wq1b.astype(ml_dtypes.bfloat16),
            wkv1b=wkv1b.astype(ml_dtypes.bfloat16),
            wq2=f32("c2_wq"), bq2=np.ascontiguousarray(f32("c2_bq")[None, :]),
            wkv2=wkv2, bkv2=bkv2,
            qw1a=np.ascontiguousarray(qw1[0:32]),
            qw1b=np.ascontiguousarray(qw1[32:160]),
            qw1c=np.ascontiguousarray(qw1[160:288]),
            qb1=f32("q_b1").reshape(128, 1),
            qw2=f32("q_w2"),
            qb2=f32("q_b2").reshape(1, 2),
            ones128=np.ones((1, 128), np.float32),
            id128=np.eye(128, dtype=np.float32),
            kv_idx=per_core_e[c]["kv_idx"],
            S_all=per_core_e[c]["S"],
            ST_all=per_core_e[c]["ST"],
            idx_x=_wrap16(idx_loc, bpad),
            own_mask=own.astype(np.float32).reshape(cfg.B, 1),
        )
        in_maps.append(im)
    return in_maps, t_lo, t_hi, chunks, base, slots


# --------------------------------------------------------------------------
# device program
# --------------------------------------------------------------------------


def build_program(cfg, t_lo, t_hi, chunks, base, slots):
    nc = bacc.Bacc("TRN2", target_bir_lowering=False, debug=False,
                   num_devices=N_CORES, num_swdge_queues=4)
    EDT = cfg.EDT
    NB, SH = cfg.NBLK, cfg.SHARD
    RG = [list(range(N_CORES))]
    RELU = mybir.ActivationFunctionType.Relu
    COPY = mybir.ActivationFunctionType.Copy
    EXP = mybir.ActivationFunctionType.Exp

    def din(name, shape, dt=F32):
        return nc.dram_tensor(name, list(shape), dt, kind="ExternalInput").ap()

    xT = din("xT", [128, SH], BF16)
    enc_w1 = din("enc_w1", [128, 32], BF16); enc_b1 = din("enc_b1", [32, 1])
    enc_w2 = din("enc_w2", [32, 32], BF16); enc_b2c = din("enc_b2c", [32, 1])
    enc_w2b = din("enc_w2b", [33, 32], BF16)
    wq1b = din("wq1b", [33, 128], BF16); wkv1b = din("wkv1b", [33, 256], BF16)
    wq2 = din("wq2", [128, 128]); bq2 = din("bq2", [1, 128])
    wkv2 = din("wkv2", [128, 256]); bkv2 = din("bkv2", [1, 256])
    qw1a = din("qw1a", [32, 128]); qw1b = din("qw1b", [128, 128])
    qw1c = din("qw1c", [128, 128]); qb1 = din("qb1", [128, 1])
    qw2 = din("qw2", [128, 2]); qb2 = din("qb2", [1, 2])
    ones128 = din("ones128", [1, 128]); id128 = din("id128", [128, 128])
    kv_idx_d = din("kv_idx", [128, slots // 16], I16)
    S_d = din("S_all", [128, slots], cfg.SDT)
    ST_d = din("ST_all", [128, slots], cfg.SDT)
    bpad = ((cfg.B + 127) // 128) * 128
    idx_x_d = din("idx_x", [128, bpad // 16], I16)
    own_mask_d = din("own_mask", [cfg.B, 1])
    out_d = nc.dram_tensor("out", [cfg.B, 2], F32, kind="ExternalOutput").ap()

    QI = cfg.QI_GROUP

    with tile.TileContext(nc) as tc:
        with (
            tc.tile_pool(name="const", bufs=1) as cpool,
            tc.tile_pool(name="kvch", bufs=3) as kvpool,
            tc.tile_pool(name="stp", bufs=5) as stpool,
            tc.tile_pool(name="sp", bufs=5) as sSpool,
            tc.tile_pool(name="qb", bufs=5) as qpool,
            tc.tile_pool(name="lt", bufs=5) as ltpool,
            tc.tile_pool(name="rhsp", bufs=4) as rhspool,
            tc.tile_pool(name="work", bufs=4) as wpool,
            tc.tile_pool(name="enc", bufs=14) as encpool,
            tc.tile_pool(name="small", bufs=3) as spool,
            tc.tile_pool(name="psQ", bufs=3, space="PSUM") as psQ,
            tc.tile_pool(name="psS", bufs=2, space="PSUM") as psS,
            tc.tile_pool(name="psT", bufs=3, space="PSUM") as psT,
            tc.tile_pool(name="dram", bufs=1, space="DRAM") as dpool,
        ):
            nc.gpsimd.load_library(library_config.mlp)

            def ld(ap, shape, dt=F32, nm=None):
                t = cpool.tile(shape, dt, name=nm or ("ld_" + ap.tensor.name))
                nc.sync.dma_start(t[:], ap[:])
                return t

            w_enc1 = ld(enc_w1, [128, 32], BF16)
            b_enc1 = ld(enc_b1, [32, 1])
            w_enc2 = ld(enc_w2, [32, 32], BF16)
            b_enc2c = ld(enc_b2c, [32, 1])
            w_enc2b = ld(enc_w2b, [33, 32], BF16)
            w_q1b = ld(wq1b, [33, 128], BF16)
            w_kv1b = ld(wkv1b, [33, 256], BF16)
            w_q2 = ld(wq2, [128, 128]); b_q2 = ld(bq2, [1, 128])
            w_kv2 = ld(wkv2, [128, 256]); b_kv2 = ld(bkv2, [1, 256])
            w_qha = ld(qw1a, [32, 128]); w_qhb = ld(qw1b, [128, 128])
            w_qhc = ld(qw1c, [128, 128])
            b_qh = ld(qb1, [128, 1]); w_qh2 = ld(qw2, [128, 2])
            b_qh2 = ld(qb2, [1, 2])
            ones_s = ld(ones128, [1, 128]); id_s = ld(id128, [128, 128])
            nidx16 = (cfg.B + 15) // 16
            idxx_s = cpool.tile([128, nidx16], I16, name="idxx_s")
            nc.sync.dma_start(idxx_s[:], idx_x_d[:, 0:nidx16])
            mask_s = ld(own_mask_d, [cfg.B, 1])
            kvidx_s = cpool.tile([128, slots // 16], I16, name="kvidx_s")
            nc.sync.dma_start(kvidx_s[:], kv_idx_d[:])

            q1_tab = dpool.tile([SH, 128], EDT, name="q1_tab")
            q2_tab = dpool.tile([SH, 128], EDT, name="q2_tab")
            kv1_sh = dpool.tile([SH, 256], EDT, name="kv1_sh")
            kv2_sh = dpool.tile([SH, 256], EDT, name="kv2_sh")
            PB = 8 * 3072
            kv1_fullA = dpool.tile([PB, 256], EDT, name="kv1_fullA",
                                   addr_space="Shared")
            kv1_fullB = dpool.tile([cfg.N_PAD - PB, 256], EDT,
                                   name="kv1_fullB", addr_space="Shared")
            kv2_fullA = dpool.tile([PB, 256], EDT, name="kv2_fullA",
                                   addr_space="Shared")
            kv2_fullB = dpool.tile([cfg.N_PAD - PB, 256], EDT,
                                   name="kv2_fullB", addr_space="Shared")
            h0_rows = dpool.tile([SH, 64], F32, name="h0_rows")
            h1_rows = dpool.tile([SH, 128], F32, name="h1_rows")
            h2_rows = dpool.tile([SH, 128], F32, name="h2_rows")
            ar_in = dpool.tile([cfg.B, 2], F32, name="ar_in")
            ar_out = dpool.tile([cfg.B, 2], F32, name="ar_out",
                                addr_space="Shared")

            # ===== encoder (own shard): kv1 rows first, then AllGather;
            # q1/h0 row emission overlaps the AllGather latency =====
            ENC_G = 4
            kept = []
            for g0 in range(0, NB, ENC_G):
                nb = min(ENC_G, NB - g0)
                W = nb * 128
                gsl = slice(g0 * 128, g0 * 128 + W)
                xch = wpool.tile([128, ENC_G * 128], BF16, tag="xch",
                                 name="xch")
                nc.sync.dma_start(xch[:, 0:W], xT[:, gsl])
                ps1 = psQ.tile([32, ENC_G * 128], F32, tag="psQ",
                               name="ps_enc1")
                nc.tensor.matmul(ps1[:, 0:W], w_enc1[:], xch[:, 0:W],
                                 start=True, stop=True)
                h1p = encpool.tile([33, ENC_G * 128], BF16, tag="h1p",
                                 name="h1p")
                nc.vector.memset(h1p[32:33, :], 1.0)
                nc.scalar.activation(h1p[0:32, 0:W], ps1[:, 0:W], RELU,
                                     bias=b_enc1[:], scale=1.0)
                ps2 = psQ.tile([32, ENC_G * 128], F32, tag="psQ",
                               name="ps_enc2")
                nc.tensor.matmul(ps2[:, 0:W], w_enc2[:], h1p[0:32, 0:W],
                                 start=True, stop=True)
                h0b = encpool.tile([33, ENC_G * 128], BF16, tag="h0b",
                                 name="h0b")
                nc.vector.memset(h0b[32:33, :], 1.0)
                nc.scalar.activation(h0b[0:32, 0:W], ps2[:, 0:W], RELU,
                                     bias=b_enc2c[:], scale=1.0)
                kept.append((g0, nb, h1p, h0b))
                for j in range(nb):
                    b = g0 + j
                    bsl = slice(b * 128, (b + 1) * 128)
                    jsl = slice(j * 128, (j + 1) * 128)
                    psk = psT.tile([128, 256], F32, tag="psT", name="ps_kv")
                    nc.tensor.matmul(psk[:], h0b[:, jsl], w_kv1b[:],
                                     start=True, stop=True)
                    kvr = spool.tile([128, 256], EDT, tag="kvr", name="kvr")
                    nc.vector.tensor_copy(kvr[:], psk[:])
                    nc.sync.dma_start(kv1_sh[bsl, :], kvr[:])
                if g0 + nb == 24:
                    # piece 0 of the table is complete: AllGather it while
                    # the encoder keeps producing the rest
                    nc.gpsimd.collective_compute(
                        "AllGather", mybir.AluOpType.bypass,
                        replica_groups=RG, ins=[kv1_sh[0:3072, :].opt()],
                        outs=[kv1_fullA.opt()])
            nc.gpsimd.collective_compute(
                "AllGather", mybir.AluOpType.bypass, replica_groups=RG,
                ins=[kv1_sh[3072:SH, :].opt()],
                outs=[kv1_fullB.opt()])
            # q1 table + h0 rows (overlaps the AllGather)
            for g0, nb, h1p, h0b in kept:
                for j in range(nb):
                    b = g0 + j
                    bsl = slice(b * 128, (b + 1) * 128)
                    jsl = slice(j * 128, (j + 1) * 128)
                    psr = psT.tile([128, 32], F32, tag="psT", name="ps_h0r")
                    nc.tensor.matmul(psr[:], h1p[:, jsl], w_enc2b[:],
                                     start=True, stop=True)
                    h0r = spool.tile([128, 64], F32, tag="h0r", name="h0r")
                    nc.scalar.activation(h0r[:, 0:32], psr[:], RELU)
                    nc.sync.dma_start(h0_rows[bsl, :], h0r[:])
                    psq = psS.tile([128, 128], F32, tag="psS", name="ps_q")
                    nc.tensor.matmul(psq[:], h0b[:, jsl], w_q1b[:],
                                     start=True, stop=True)
                    qr = spool.tile([128, 128], EDT, tag="qr", name="qr")
                    nc.scalar.activation(qr[:], psq[:], COPY)
                    nc.sync.dma_start(q1_tab[bsl, :], qr[:])

            # ================= conv layers =================
            PREP_AHEAD = 0

            def emit_conv(q_tab, kv_tabs, h_rows_out, emit_tab2, lname,
                          mid_cb=None):
                qrr = [0]

                def next_q():
                    q = qrr[0] & 3
                    qrr[0] += 1
                    return q

                def emit_gathers(ch, kv_ch, prep):
                    tile0 = ch["tile0"]
                    for b in ch["blocks"]:
                        for h in (0, 1):
                            nt = (t_hi if h else t_lo)[b]
                            toff = int(base[b]) - tile0 + (t_lo[b] if h else 0)
                            c8 = (int(base[b]) + (t_lo[b] if h else 0)) * 8
                            kw = {}
                            if prep:
                                kw = dict(prepare_only=True,
                                          sem=nc.alloc_semaphore(
                                              f"pg_{lname}_{b}_{h}"))
                            nc.gpsimd.dma_gather(
                                kv_ch[:, toff: toff + nt, :],
                                kv_tabs[h][:, :],
                                kvidx_s[:, c8: c8 + nt * 8], nt * 128,
                                nt * 128, 256, single_packet=False,
                                queue_num=next_q(), **kw)

                # descriptor pre-generation for the first chunks: desc-gen
                # runs while the kv AllGather is still in flight; the
                # triggers (which carry the kv_full data dep) fire the DMAs
                # the moment the table lands.
                prep_tiles = []
                for ch in chunks[:PREP_AHEAD]:
                    kv_ch = kvpool.tile([128, ch["tiles"], 256], EDT,
                                        tag="kv_ch", name="kv_ch")
                    emit_gathers(ch, kv_ch, True)
                    prep_tiles.append(kv_ch)
                if prep_tiles:
                    for q in range(4):
                        nc.gpsimd.trigger_dma(count=None, queue_num=q)

                for ci, ch in enumerate(chunks):
                    blocks = ch["blocks"]
                    TC = ch["tiles"]
                    tile0 = ch["tile0"]

                    if ci < PREP_AHEAD:
                        kv_ch = prep_tiles[ci]
                    else:
                        kv_ch = kvpool.tile([128, TC, 256], EDT, tag="kv_ch",
                                            name="kv_ch")
                        emit_gathers(ch, kv_ch, False)

                    st_ts, s_ts, q_bs, l_ts, rhs_ts = {}, {}, {}, {}, {}
                    for b in blocks:
                        T = t_lo[b] + t_hi[b]
                        g0 = int(base[b]) * 128
                        st_t = stpool.tile([128, T * 128], cfg.SDT,
                                           tag="ST_b", name="ST_b")
                        nc.sync.dma_start(st_t[:], ST_d[:, g0: g0 + T * 128])
                        st_ts[b] = st_t
                        s_t = sSpool.tile([128, T * 128], cfg.SDT,
                                          tag="S_b", name="S_b")
                        nc.sync.dma_start(s_t[:], S_d[:, g0: g0 + T * 128])
                        s_ts[b] = s_t
                        bsl = slice(b * 128, (b + 1) * 128)
                        q_blk = qpool.tile([128, 128], EDT, tag="q_blk",
                                           name="q_blk")
                        nc.sync.dma_start(q_blk[:], q_tab[bsl, :])
                        q_bs[b] = q_blk

                    # stage 1: qi matmuls + logits
                    for b in blocks:
                        T = t_lo[b] + t_hi[b]
                        toff = int(base[b]) - tile0
                        st_t = st_ts[b]
                        l_t = ltpool.tile([128, T * 4], F32, tag="l_t",
                                          name="l_t")
                        l_ts[b] = l_t
                        for gs in range(0, T, QI):
                            g = min(QI, T - gs)
                            qi_ps = psQ.tile([128, QI, 128], F32, tag="psQ",
                                             name="qi_ps")
                            for i in range(g):
                                c0 = (gs + i) * 128
                                nc.tensor.matmul(
                                    qi_ps[:, i, :],
                                    st_t[:, c0: c0 + 128], q_bs[b][:],
                                    start=True, stop=True)
                            prod = spool.tile([128, QI, 128], EDT,
                                              tag="prod", name="prod")
                            nc.vector.tensor_tensor(
                                prod[:, 0:g, :],
                                qi_ps[:, 0:g, :],
                                kv_ch[:, toff + gs: toff + gs + g, 0:128],
                                mybir.AluOpType.mult)
                            nc.vector.tensor_reduce(
                                l_t[:, gs * 4: (gs + g) * 4].rearrange(
                                    "p (t h) -> p t h", h=4),
                                prod[:, 0:g, :].rearrange(
                                    "p t (h j) -> p t h j", h=4, j=32),
                                mybir.AxisListType.X, mybir.AluOpType.add)

                    # stage 2: exp + v*alpha
                    veng = nc.vector
                    for b in blocks:
                        T = t_lo[b] + t_hi[b]
                        toff = int(base[b]) - tile0
                        rhs = rhspool.tile([128, T, 132], EDT, tag="rhs",
                                           name="rhs")
                        rhs_ts[b] = rhs
                        nc.scalar.activation(
                            rhs[:, :, 0:4],
                            l_ts[b][:].rearrange("p (t h) -> p t h", h=4),
                            EXP, scale=float(cfg.SCALE))
                        veng.tensor_tensor(
                            rhs[:, :, 4:132].rearrange(
                                "p t (h j) -> p t h j", h=4, j=32),
                            kv_ch[:, toff: toff + T, 128:256].rearrange(
                                "p t (h j) -> p t h j", h=4, j=32),
                            rhs[:, :, 0:4].unsqueeze(-1).broadcast_to(
                                [128, T, 4, 32]),
                            mybir.AluOpType.mult)

                    # stage 3: aggregate + normalize + output (+ tab2)
                    for b in blocks:
                        T = t_lo[b] + t_hi[b]
                        bsl = slice(b * 128, (b + 1) * 128)
                        s_t = s_ts[b]
                        rhs = rhs_ts[b]
                        sc_ps = psS.tile([128, 132], F32, tag="psS",
                                         name="sc_ps")
                        for t in range(T):
                            nc.tensor.matmul(
                                sc_ps[:], s_t[:, t * 128: (t + 1) * 128],
                                rhs[:, t, :], start=(t == 0),
                                stop=(t == T - 1))

                        den = spool.tile([128, 4], F32, tag="den", name="den")
                        nc.vector.tensor_scalar_add(den[:], sc_ps[:, 0:4],
                                                    float(cfg.EPS))
                        rec = spool.tile([128, 4], F32, tag="rec", name="rec")
                        nc.vector.reciprocal(rec[:], den[:])
                        h_tmp = spool.tile([128, 128], F32, tag="h_tmp",
                                           name="h_tmp")
                        nc.vector.tensor_tensor(
                            h_tmp[:].rearrange("p (h j) -> p h j", h=4, j=32),
                            sc_ps[:, 4:132].rearrange(
                                "p (h j) -> p h j", h=4, j=32),
                            rec[:].unsqueeze(-1).broadcast_to([128, 4, 32]),
                            mybir.AluOpType.mult)
                        h_blk = spool.tile([128, 128], F32, tag="h_blk",
                                           name="h_blk")
                        nc.scalar.activation(h_blk[:], h_tmp[:], RELU)
                        nc.sync.dma_start(h_rows_out[bsl, :], h_blk[:])
                        if emit_tab2:
                            tr_ps = psT.tile([128, 128], F32, tag="psT",
                                             name="tr_ps")
                            nc.tensor.transpose(tr_ps[:], h_blk[:], id_s[:])
                            h1tb = spool.tile([128, 128], F32, tag="h1tb",
                                              name="h1tb")
                            nc.scalar.activation(h1tb[:], tr_ps[:], COPY)
                            # conv2 q/kv table rows, inline during conv1
                            psq = psS.tile([128, 128], F32, tag="psS",
                                           name="ps_q2")
                            nc.tensor.matmul(psq[:], h1tb[:], w_q2[:],
                                             start=True, stop=False)
                            nc.tensor.matmul(psq[:], ones_s[:], b_q2[:],
                                             start=False, stop=True)
                            qr = spool.tile([128, 128], EDT, tag="qr",
                                            name="qr2")
                            nc.scalar.activation(qr[:], psq[:], COPY)
                            nc.sync.dma_start(q2_tab[bsl, :], qr[:])
                            psk = psT.tile([128, 256], F32, tag="psT",
                                           name="ps_kv2")
                            nc.tensor.matmul(psk[:], h1tb[:], w_kv2[:],
                                             start=True, stop=False)
                            nc.tensor.matmul(psk[:], ones_s[:], b_kv2[:],
                                             start=False, stop=True)
                            kvr = spool.tile([128, 256], EDT, tag="kvr",
                                             name="kvr2")
                            nc.vector.tensor_copy(kvr[:], psk[:])
                            nc.sync.dma_start(kv2_sh[bsl, :], kvr[:])
                    if mid_cb is not None:
                        mid_cb(blocks[-1] + 1)

            ag_state = {"fired": False}

            def kv2_mid(blocks_done):
                if not ag_state["fired"] and blocks_done >= 24:
                    ag_state["fired"] = True
                    nc.gpsimd.collective_compute(
                        "AllGather", mybir.AluOpType.bypass,
                        replica_groups=RG, ins=[kv2_sh[0:3072, :].opt()],
                        outs=[kv2_fullA.opt()])

            emit_conv(q1_tab, (kv1_fullA, kv1_fullB), h1_rows, True,
                      'c1', mid_cb=kv2_mid)
            nc.gpsimd.collective_compute(
                "AllGather", mybir.AluOpType.bypass, replica_groups=RG,
                ins=[kv2_sh[3072:SH, :].opt()],
                outs=[kv2_fullB.opt()])

            # ================= Q head =================
            def gather_xT(tab, width):
                g = spool.tile([128, 1, width], F32, tag="gx", name="gx")
                nc.gpsimd.dma_gather(g[:], tab[:, :], idxx_s[:],
                                     cfg.B, cfg.B, width)
                tp = psT.tile([128, 128], F32, tag="psT", name="tp_x")
                nc.tensor.transpose(tp[0:width, 0: cfg.B], g[0: cfg.B, 0, :],
                                    id_s[0: cfg.B, 0: cfg.B])
                xt = spool.tile([128, cfg.B], F32, tag="xt", name="xt")
                nc.scalar.activation(xt[0:width, :], tp[0:width, 0: cfg.B],
                                     COPY)
                return xt

            # x1/x2 part (h0/h1 ready once conv1 finished; overlaps conv2)
            x1t = gather_xT(h0_rows, 64)
            x2t = gather_xT(h1_rows, 128)
            zh12_ps = psS.tile([128, cfg.B], F32, tag="psS", name="zh12_ps")
            nc.tensor.matmul(zh12_ps[:], w_qha[:], x1t[0:32, :],
                             start=True, stop=False)
            nc.tensor.matmul(zh12_ps[:], w_qhb[:], x2t[0:128, :],
                             start=False, stop=True)
            zh12 = spool.tile([128, cfg.B], F32, tag="zh12", name="zh12")
            nc.scalar.activation(zh12[:], zh12_ps[:], COPY)

            emit_conv(q2_tab, (kv2_fullA, kv2_fullB), h2_rows, False,
                      'c2')

            x3t = gather_xT(h2_rows, 128)
            zh_ps = psQ.tile([128, cfg.B], F32, tag="psQ", name="zh_ps")
            nc.tensor.matmul(zh_ps[:], w_qhc[:], x3t[0:128, :],
                             start=True, stop=True)
            zh = spool.tile([128, cfg.B], F32, tag="zh", name="zh")
            nc.vector.tensor_tensor(zh[:], zh_ps[:], zh12[:],
                                    mybir.AluOpType.add)
            zhr = spool.tile([128, cfg.B], F32, tag="zhr", name="zhr")
            nc.scalar.activation(zhr[:], zh[:], RELU, bias=b_qh[:],
                                 scale=1.0)
            o_ps = psS.tile([cfg.B, 2], F32, tag="psS", name="o_ps")
            nc.tensor.matmul(o_ps[:], zhr[:], w_qh2[:], start=True,
                             stop=False)
            nc.tensor.matmul(o_ps[:], ones_s[:, 0: cfg.B], b_qh2[:],
                             start=False, stop=True)
            ob = spool.tile([cfg.B, 2], F32, tag="ob", name="ob")
            nc.vector.tensor_scalar_mul(ob[:], o_ps[:], mask_s[:])
            nc.sync.dma_start(ar_in[:, :], ob[:])
            nc.gpsimd.collective_compute(
                "AllReduce", mybir.AluOpType.add, replica_groups=RG,
                ins=[ar_in.opt()], outs=[ar_out.opt()])
            nc.sync.dma_start(out_d[:, :], ar_out[:, :])

    nc.compile()
    return nc


# --------------------------------------------------------------------------
# entry point
# --------------------------------------------------------------------------

_trace_flag = {"trace": False}
_last = {}


def _chunk_key(chunks):
    return tuple((tuple(ch["blocks"]), ch["tiles"], ch["tile0"])
                 for ch in chunks)


def _run(inputs, cfg=None):
    cfg = cfg or Cfg()
    in_maps, t_lo, t_hi, chunks, base, slots = _prep_inputs(cfg, inputs)
    key = (slots, tuple(t_lo), tuple(t_hi), _chunk_key(chunks), cfg.edge_bf16)
    if _last.get("key") != key:
        _last["nc"] = build_program(cfg, t_lo, t_hi, chunks, base, slots)
        _last["key"] = key
    nc = _last["nc"]
    res = bass_utils.run_bass_kernel_spmd(
        nc, in_maps, core_ids=list(range(N_CORES)),
        trace=_trace_flag["trace"])
    _last["res"] = res
    return res.results[0]["out"].astype(np.float32)


def kernel(**inputs):
    return _run(inputs)


# revision 22
# speedup vs baseline: 1.5657x; 1.1782x over previous
"""Trainium2 Bass kernel for DGNRNetwork (2-layer TransformerConv GNN + MLPs).

Strategy (8 NeuronCores, graph/data parallel):
  - Nodes padded to N_PAD=50176 and sharded by contiguous range: core c owns
    nodes [c*6272, (c+1)*6272), i.e. 49 blocks of 128 dst nodes per core.
  - Edges partitioned by dst shard on host, laid out BLOCK-MAJOR: for each
    dst block, its lo-half slots then hi-half slots (each padded to whole
    128-edge tiles, uniform across cores -> one SPMD program).  Each block's
    slots are contiguous, so S / S_T / kv_ch slices are contiguous and one
    DMA per block suffices.
  - k||v rows fetched with one indirect DMA (dma_gather) per (block, half),
    round-robined over the 4 SWDGE queues so descriptor generation runs
    concurrently on the 4 GpSimd core pairs.
  - qi = S_T_tile @ Q_blk on TensorE (host-precomputed one-hot S_T).
  - Per-edge logits on Vector (qi straight from PSUM); exp on Scalar into
    rhs[:, :, 0:4]; the v*alpha product reads the exp'd logits broadcast
    (stride-0) so no materialized [128,T,128] attention tile.
  - Segment softmax denominator and weighted sum are ONE accumulated TensorE
    matmul chain with the one-hot scatter matrix S.  Padding edges have
    all-zero S rows so they drop out.
  - Conv loop is stage-batched per chunk (stage1 qi+logits for all blocks,
    stage2 exp+rhs, stage3 aggregate+finish) so the in-order engines overlap
    across blocks instead of ping-ponging inside one block's serial chain.
  - Small weights replicated; kv tables exchanged with AllGather between
    layers; tiny Q-head computed redundantly, combined with masked AllReduce.
"""

import sys

sys.path.insert(0, "/opt/trn_rl_repo")

import numpy as np
import ml_dtypes

import concourse.bacc as bacc
import concourse.bass as bass
import concourse.mybir as mybir
import concourse.tile as tile
from concourse import bass_utils, library_config

F32 = mybir.dt.float32
BF16 = mybir.dt.bfloat16
I16 = mybir.dt.int16

N_CORES = 8


class Cfg:
    def __init__(self, n_nodes=50000, nblk=49, b=64, edge_bf16=True,
                 chunk_tiles=40, qi_group=4):
        self.N = n_nodes
        self.NBLK = nblk                 # dst blocks per core
        self.SHARD = nblk * 128          # nodes per core
        self.N_PAD = 8 * self.SHARD
        self.HALF = self.N_PAD // 2      # kv table split (int16 gather idx)
        self.B = b                       # batch (selected nodes)
        self.F_IN = 128
        self.H = 32
        self.HD = 128
        self.HEADS = 4
        self.EPS = 1e-16
        self.SCALE = 1.0 / np.sqrt(32.0)
        self.edge_bf16 = edge_bf16
        self.EDT = BF16 if edge_bf16 else F32
        self.EDT_NP = ml_dtypes.bfloat16 if edge_bf16 else np.float32
        self.s_fp8 = edge_bf16
        self.SDT = mybir.dt.float8e4 if self.s_fp8 else self.EDT
        self.SDT_NP = ml_dtypes.float8_e4m3 if self.s_fp8 else self.EDT_NP
        self.CHUNK_TILES = chunk_tiles   # max 128-edge tiles per chunk
        self.QI_GROUP = qi_group         # tiles per qi PSUM group (1 bank)
        assert self.N <= self.N_PAD and self.HALF < 32768


# --------------------------------------------------------------------------
# host-side preprocessing
# --------------------------------------------------------------------------


def _wrap16(values, slots):
    """dma_gather idx layout: idx i lives at [i % 16, i // 16], replicated
    across the eight 16-partition groups."""
    arr = np.zeros((16, slots // 16), dtype=np.int16)
    arr[np.arange(len(values)) % 16, np.arange(len(values)) // 16] = values
    return np.tile(arr, (8, 1))


def _plan_chunks(cfg, t_lo, t_hi):
    """Group consecutive blocks into chunks of <= CHUNK_TILES tiles.

    Block-major layout: block b occupies tiles [base[b], base[b]+t_lo[b]+
    t_hi[b]) -- lo tiles then hi tiles, contiguous.  Returns chunks: list of
    dicts with blocks, tile0 (global tile index of chunk start), tiles.
    """
    base = np.zeros(cfg.NBLK + 1, np.int64)
    for b in range(cfg.NBLK):
        base[b + 1] = base[b] + t_lo[b] + t_hi[b]
    chunks = []
    b = 0
    while b < cfg.NBLK:
        blocks = []
        tl = 0
        while b < cfg.NBLK:
            need = t_lo[b] + t_hi[b]
            if blocks and tl + need > cfg.CHUNK_TILES:
                break
            blocks.append(b)
            tl += need
            b += 1
        chunks.append(dict(blocks=blocks, tile0=int(base[blocks[0]]),
                           tiles=tl))
    return chunks, base, int(base[cfg.NBLK])




def _balance_perm(cfg, edge_index):
    """Per-core node->block packing: best-fit-decreasing into blocks capped
    at 1024 in-edges per src half (8 tiles), overflow concentrated in the
    trailing blocks.  Cores keep their node ranges (src lo/hi halves
    invariant); only positions within each shard permute.  Returns
    pos_of_old: old node id -> new node id."""
    esrc = np.asarray(edge_index[0]).astype(np.int64)
    edst = np.asarray(edge_index[1]).astype(np.int64)
    hi = (esrc >= cfg.HALF).astype(np.int64)
    deg = np.zeros((cfg.N_PAD, 2), np.int64)
    np.add.at(deg, (edst, hi), 1)

    CAP = 1024
    K = 3
    pos_of_old = np.zeros(cfg.N_PAD, np.int64)
    nb = cfg.NBLK
    nreg = nb - K
    for c in range(N_CORES):
        ids = np.arange(c * cfg.SHARD, (c + 1) * cfg.SHARD)
        dl = deg[ids, 0]
        dh = deg[ids, 1]
        order = np.argsort(-(dl + dh), kind="stable")
        fl = np.zeros(nb, np.int64)
        fh = np.zeros(nb, np.int64)
        cnt = np.zeros(nb, np.int64)
        assign = np.full(len(ids), -1, np.int64)
        leftover = []
        for i in order:
            ok = ((cnt[:nreg] < 128) & (fl[:nreg] + dl[i] <= CAP)
                  & (fh[:nreg] + dh[i] <= CAP))
            if ok.any():
                cand = np.where(ok)[0]
                score = np.maximum(fl[cand] + dl[i], fh[cand] + dh[i])
                b = int(cand[np.argmax(score)])
            else:
                leftover.append(i)
                continue
            assign[i] = b
            fl[b] += dl[i]
            fh[b] += dh[i]
            cnt[b] += 1
        leftover.sort(key=lambda i: -(dl[i] + dh[i]))
        for i in leftover:
            room = np.where(cnt < 128)[0]
            cand = room[room >= nreg] if (room >= nreg).any() else room
            b = int(cand[np.argmin(np.maximum(fl[cand] + dl[i],
                                              fh[cand] + dh[i]))])
            assign[i] = b
            fl[b] += dl[i]
            fh[b] += dh[i]
            cnt[b] += 1
        binorder = np.lexsort((np.arange(len(ids)), assign))
        rank = np.zeros(len(ids), np.int64)
        pos = np.zeros(nb, np.int64)
        for i in binorder:
            b = assign[i]
            rank[i] = pos[b]
            pos[b] += 1
        pos_of_old[ids] = c * cfg.SHARD + assign * 128 + rank
    return pos_of_old

def _prep_edges(cfg, edge_index):
    src = np.ascontiguousarray(edge_index[0]).astype(np.int64)
    dst = np.ascontiguousarray(edge_index[1]).astype(np.int64)
    core = dst // cfg.SHARD
    blk = (dst % cfg.SHARD) // 128
    hi = (src >= cfg.HALF).astype(np.int64)

    cnt = np.zeros((N_CORES, cfg.NBLK, 2), np.int64)
    np.add.at(cnt, (core, blk, hi), 1)
    t_lo = np.maximum(1, (cnt[:, :, 0].max(0) + 127) // 128)  # [NBLK]
    t_hi = np.maximum(1, (cnt[:, :, 1].max(0) + 127) // 128)

    chunks, base, total_tiles = _plan_chunks(cfg, t_lo.tolist(), t_hi.tolist())
    slots = total_tiles * 128
    assert slots % 16 == 0

    # global slot base for each (blk, hi) group (block-major layout)
    grp_base = np.zeros((cfg.NBLK, 2), np.int64)
    for b in range(cfg.NBLK):
        grp_base[b, 0] = base[b] * 128
        grp_base[b, 1] = (base[b] + t_lo[b]) * 128

    order = np.lexsort((src, hi, blk, core))
    s_src, s_dst, s_core, s_blk, s_hi = (
        src[order], dst[order], core[order], blk[order], hi[order])

    per_core = []
    for c in range(N_CORES):
        m = s_core == c
        csrc, cdst, cblk, chi = s_src[m], s_dst[m], s_blk[m], s_hi[m]
        # composite key non-decreasing under the sort above
        key = cblk * 2 + chi
        kcounts = np.bincount(key, minlength=cfg.NBLK * 2)
        kstarts = np.zeros_like(kcounts)
        kstarts[1:] = np.cumsum(kcounts)[:-1]
        rank = np.arange(len(key)) - kstarts[key]
        slot = grp_base[cblk, chi] + rank

        kv_val = np.where(chi == 1, csrc - cfg.HALF, csrc)
        kv_idx = np.zeros(slots, np.int64)
        kv_idx[slot] = kv_val

        S = np.zeros((128, slots), cfg.SDT_NP)
        scol = (slot // 128) * 128 + (cdst % 128)
        S[slot % 128, scol] = 1.0
        ST = np.zeros((128, slots), cfg.SDT_NP)
        stcol = (slot // 128) * 128 + (slot % 128)
        ST[cdst % 128, stcol] = 1.0

        per_core.append(dict(kv_idx=_wrap16(kv_idx, slots), S=S, ST=ST))
    return per_core, t_lo.tolist(), t_hi.tolist(), chunks, base, slots


def _prep_inputs(cfg, inputs):
    x = np.asarray(inputs["x"], np.float32)
    idx = np.asarray(inputs["idx"]).astype(np.int64)
    f32 = lambda k: np.ascontiguousarray(np.asarray(inputs[k], np.float32))

    ei = np.asarray(inputs["edge_index"]).astype(np.int64)
    pos_of_old = _balance_perm(cfg, ei)
    ei = pos_of_old[ei]
    idx = pos_of_old[idx]

    xp = np.zeros((cfg.N_PAD, cfg.F_IN), np.float32)
    old_of_new = np.argsort(pos_of_old)
    sel = old_of_new[old_of_new < cfg.N]
    xp[pos_of_old[sel]] = x[sel]

    per_core_e, t_lo, t_hi, chunks, base, slots = _prep_edges(cfg, ei)

    wkv1b = np.ascontiguousarray(np.concatenate([
        np.concatenate([f32("c1_wk"), f32("c1_wv")], axis=1),
        np.concatenate([f32("c1_bk"), f32("c1_bv")])[None, :]], axis=0))
    wq1b = np.ascontiguousarray(
        np.concatenate([f32("c1_wq"), f32("c1_bq")[None, :]], axis=0))
    enc_w2b = np.ascontiguousarray(
        np.concatenate([f32("enc_w2"), f32("enc_b2")[None, :]], axis=0))
    wkv2 = np.ascontiguousarray(
        np.concatenate([f32("c2_wk"), f32("c2_wv")], axis=1))     # [128,256]
    bkv2 = np.ascontiguousarray(
        np.concatenate([f32("c2_bk"), f32("c2_bv")])[None, :])
    qw1 = f32("q_w1")                                              # [288,128]
    bpad = ((cfg.B + 127) // 128) * 128

    in_maps = []
    for c in range(N_CORES):
        shard = slice(c * cfg.SHARD, (c + 1) * cfg.SHARD)
        own = (idx // cfg.SHARD) == c
        idx_loc = np.where(own, idx - c * cfg.SHARD, 0)
        im = dict(
            xT=np.ascontiguousarray(
                xp[shard].T.astype(ml_dtypes.bfloat16)),   # [128, SHARD]
            enc_w1=f32("enc_w1").astype(ml_dtypes.bfloat16),
            enc_b1=f32("enc_b1").reshape(32, 1),
            enc_w2=f32("enc_w2").astype(ml_dtypes.bfloat16),
            enc_b2c=f32("enc_b2").reshape(32, 1),
            enc_w2b=enc_w2b.astype(ml_dtypes.bfloat16),
            wq1b=wq1b.astype(ml_dtypes.bfloat16),
            wkv1b=wkv1b.astype(ml_dtypes.bfloat16),
            wq2=f32("c2_wq"), bq2=np.ascontiguousarray(f32("c2_bq")[None, :]),
            wkv2=wkv2, bkv2=bkv2,
            qw1a=np.ascontiguousarray(qw1[0:32]),
            qw1b=np.ascontiguousarray(qw1[32:160]),
            qw1c=np.ascontiguousarray(qw1[160:288]),
            qb1=f32("q_b1").reshape(128, 1),
            qw2=f32("q_w2"),
            qb2=f32("q_b2").reshape(1, 2),
            ones128=np.ones((1, 128), np.float32),
            id128=np.eye(128, dtype=np.float32),
            kv_idx=per_core_e[c]["kv_idx"],
            S_all=per_core_e[c]["S"],
            ST_all=per_core_e[c]["ST"],
            idx_x=_wrap16(idx_loc, bpad),
            own_mask=own.astype(np.float32).reshape(cfg.B, 1),
        )
        in_maps.append(im)
    return in_maps, t_lo, t_hi, chunks, base, slots


# --------------------------------------------------------------------------
# device program
# --------------------------------------------------------------------------


def build_program(cfg, t_lo, t_hi, chunks, base, slots):
    nc = bacc.Bacc("TRN2", target_bir_lowering=False, debug=False,
                   num_devices=N_CORES, num_swdge_queues=4)
    EDT = cfg.EDT
    NB, SH = cfg.NBLK, cfg.SHARD
    RG = [list(range(N_CORES))]
    RELU = mybir.ActivationFunctionType.Relu
    COPY = mybir.ActivationFunctionType.Copy
    EXP = mybir.ActivationFunctionType.Exp

    def din(name, shape, dt=F32):
        return nc.dram_tensor(name, list(shape), dt, kind="ExternalInput").ap()

    xT = din("xT", [128, SH], BF16)
    enc_w1 = din("enc_w1", [128, 32], BF16); enc_b1 = din("enc_b1", [32, 1])
    enc_w2 = din("enc_w2", [32, 32], BF16); enc_b2c = din("enc_b2c", [32, 1])
    enc_w2b = din("enc_w2b", [33, 32], BF16)
    wq1b = din("wq1b", [33, 128], BF16); wkv1b = din("wkv1b", [33, 256], BF16)
    wq2 = din("wq2", [128, 128]); bq2 = din("bq2", [1, 128])
    wkv2 = din("wkv2", [128, 256]); bkv2 = din("bkv2", [1, 256])
    qw1a = din("qw1a", [32, 128]); qw1b = din("qw1b", [128, 128])
    qw1c = din("qw1c", [128, 128]); qb1 = din("qb1", [128, 1])
    qw2 = din("qw2", [128, 2]); qb2 = din("qb2", [1, 2])
    ones128 = din("ones128", [1, 128]); id128 = din("id128", [128, 128])
    kv_idx_d = din("kv_idx", [128, slots // 16], I16)
    S_d = din("S_all", [128, slots], cfg.SDT)
    ST_d = din("ST_all", [128, slots], cfg.SDT)
    bpad = ((cfg.B + 127) // 128) * 128
    idx_x_d = din("idx_x", [128, bpad // 16], I16)
    own_mask_d = din("own_mask", [cfg.B, 1])
    out_d = nc.dram_tensor("out", [cfg.B, 2], F32, kind="ExternalOutput").ap()

    QI = cfg.QI_GROUP

    with tile.TileContext(nc) as tc:
        with (
            tc.tile_pool(name="const", bufs=1) as cpool,
            tc.tile_pool(name="kvch", bufs=4) as kvpool,
            tc.tile_pool(name="stp", bufs=5) as stpool,
            tc.tile_pool(name="sp", bufs=5) as sSpool,
            tc.tile_pool(name="qb", bufs=5) as qpool,
            tc.tile_pool(name="lt", bufs=5) as ltpool,
            tc.tile_pool(name="rhsp", bufs=5) as rhspool,
            tc.tile_pool(name="work", bufs=4) as wpool,
            tc.tile_pool(name="enc", bufs=14) as encpool,
            tc.tile_pool(name="small", bufs=3) as spool,
            tc.tile_pool(name="psQ", bufs=3, space="PSUM") as psQ,
            tc.tile_pool(name="psS", bufs=2, space="PSUM") as psS,
            tc.tile_pool(name="psT", bufs=3, space="PSUM") as psT,
            tc.tile_pool(name="dram", bufs=1, space="DRAM") as dpool,
        ):
            nc.gpsimd.load_library(library_config.mlp)

            def ld(ap, shape, dt=F32, nm=None):
                t = cpool.tile(shape, dt, name=nm or ("ld_" + ap.tensor.name))
                nc.sync.dma_start(t[:], ap[:])
                return t

            w_enc1 = ld(enc_w1, [128, 32], BF16)
            b_enc1 = ld(enc_b1, [32, 1])
            w_enc2 = ld(enc_w2, [32, 32], BF16)
            b_enc2c = ld(enc_b2c, [32, 1])
            w_enc2b = ld(enc_w2b, [33, 32], BF16)
            w_q1b = ld(wq1b, [33, 128], BF16)
            w_kv1b = ld(wkv1b, [33, 256], BF16)
            w_q2 = ld(wq2, [128, 128]); b_q2 = ld(bq2, [1, 128])
            w_kv2 = ld(wkv2, [128, 256]); b_kv2 = ld(bkv2, [1, 256])
            w_qha = ld(qw1a, [32, 128]); w_qhb = ld(qw1b, [128, 128])
            w_qhc = ld(qw1c, [128, 128])
            b_qh = ld(qb1, [128, 1]); w_qh2 = ld(qw2, [128, 2])
            b_qh2 = ld(qb2, [1, 2])
            ones_s = ld(ones128, [1, 128]); id_s = ld(id128, [128, 128])
            nidx16 = (cfg.B + 15) // 16
            idxx_s = cpool.tile([128, nidx16], I16, name="idxx_s")
            nc.sync.dma_start(idxx_s[:], idx_x_d[:, 0:nidx16])
            mask_s = ld(own_mask_d, [cfg.B, 1])
            kvidx_s = cpool.tile([128, slots // 16], I16, name="kvidx_s")
            nc.sync.dma_start(kvidx_s[:], kv_idx_d[:])

            q1_tab = dpool.tile([SH, 128], EDT, name="q1_tab")
            q2_tab = dpool.tile([SH, 128], EDT, name="q2_tab")
            kv1_sh = dpool.tile([SH, 256], EDT, name="kv1_sh")
            kv2_sh = dpool.tile([SH, 256], EDT, name="kv2_sh")
            kv1_full = dpool.tile([cfg.N_PAD, 256], EDT, name="kv1_full",
                                  addr_space="Shared")
            kv2_full = dpool.tile([cfg.N_PAD, 256], EDT, name="kv2_full",
                                  addr_space="Shared")
            h0_rows = dpool.tile([SH, 64], F32, name="h0_rows")
            h1_rows = dpool.tile([SH, 128], F32, name="h1_rows")
            h2_rows = dpool.tile([SH, 128], F32, name="h2_rows")
            ar_in = dpool.tile([cfg.B, 2], F32, name="ar_in")
            ar_out = dpool.tile([cfg.B, 2], F32, name="ar_out",
                                addr_space="Shared")

            # ===== encoder (own shard): kv1 rows first, then AllGather;
            # q1/h0 row emission overlaps the AllGather latency =====
            ENC_G = 4
            kept = []
            for g0 in range(0, NB, ENC_G):
                nb = min(ENC_G, NB - g0)
                W = nb * 128
                gsl = slice(g0 * 128, g0 * 128 + W)
                xch = wpool.tile([128, ENC_G * 128], BF16, tag="xch",
                                 name="xch")
                nc.sync.dma_start(xch[:, 0:W], xT[:, gsl])
                ps1 = psQ.tile([32, ENC_G * 128], F32, tag="psQ",
                               name="ps_enc1")
                nc.tensor.matmul(ps1[:, 0:W], w_enc1[:], xch[:, 0:W],
                                 start=True, stop=True)
                h1p = encpool.tile([33, ENC_G * 128], BF16, tag="h1p",
                                 name="h1p")
                nc.vector.memset(h1p[32:33, :], 1.0)
                nc.scalar.activation(h1p[0:32, 0:W], ps1[:, 0:W], RELU,
                                     bias=b_enc1[:], scale=1.0)
                ps2 = psQ.tile([32, ENC_G * 128], F32, tag="psQ",
                               name="ps_enc2")
                nc.tensor.matmul(ps2[:, 0:W], w_enc2[:], h1p[0:32, 0:W],
                                 start=True, stop=True)
                h0b = encpool.tile([33, ENC_G * 128], BF16, tag="h0b",
                                 name="h0b")
                nc.vector.memset(h0b[32:33, :], 1.0)
                nc.scalar.activation(h0b[0:32, 0:W], ps2[:, 0:W], RELU,
                                     bias=b_enc2c[:], scale=1.0)
                kept.append((g0, nb, h1p, h0b))
                for j in range(nb):
                    b = g0 + j
                    bsl = slice(b * 128, (b + 1) * 128)
                    jsl = slice(j * 128, (j + 1) * 128)
                    psk = psT.tile([128, 256], F32, tag="psT", name="ps_kv")
                    nc.tensor.matmul(psk[:], h0b[:, jsl], w_kv1b[:],
                                     start=True, stop=True)
                    kvr = spool.tile([128, 256], EDT, tag="kvr", name="kvr")
                    nc.vector.tensor_copy(kvr[:], psk[:])
                    nc.sync.dma_start(kv1_sh[bsl, :], kvr[:])
            nc.gpsimd.collective_compute(
                "AllGather", mybir.AluOpType.bypass, replica_groups=RG,
                ins=[kv1_sh.opt()], outs=[kv1_full.opt()])
            # q1 table + h0 rows (overlaps the AllGather)
            for g0, nb, h1p, h0b in kept:
                for j in range(nb):
                    b = g0 + j
                    bsl = slice(b * 128, (b + 1) * 128)
                    jsl = slice(j * 128, (j + 1) * 128)
                    psr = psT.tile([128, 32], F32, tag="psT", name="ps_h0r")
                    nc.tensor.matmul(psr[:], h1p[:, jsl], w_enc2b[:],
                                     start=True, stop=True)
                    h0r = spool.tile([128, 64], F32, tag="h0r", name="h0r")
                    nc.scalar.activation(h0r[:, 0:32], psr[:], RELU)
                    nc.sync.dma_start(h0_rows[bsl, :], h0r[:])
                    psq = psS.tile([128, 128], F32, tag="psS", name="ps_q")
                    nc.tensor.matmul(psq[:], h0b[:, jsl], w_q1b[:],
                                     start=True, stop=True)
                    qr = spool.tile([128, 128], EDT, tag="qr", name="qr")
                    nc.scalar.activation(qr[:], psq[:], COPY)
                    nc.sync.dma_start(q1_tab[bsl, :], qr[:])

            # ================= conv layers =================
            PREP_AHEAD = 0

            def emit_conv(q_tab, kv_full, h_rows_out, emit_tab2, lname):
                qrr = [0]

                def next_q():
                    q = qrr[0] & 3
                    qrr[0] += 1
                    return q

                def emit_gathers(ch, kv_ch, prep):
                    tile0 = ch["tile0"]
                    for b in ch["blocks"]:
                        for h in (0, 1):
                            nt = (t_hi if h else t_lo)[b]
                            toff = int(base[b]) - tile0 + (t_lo[b] if h else 0)
                            c8 = (int(base[b]) + (t_lo[b] if h else 0)) * 8
                            kw = {}
                            if prep:
                                kw = dict(prepare_only=True,
                                          sem=nc.alloc_semaphore(
                                              f"pg_{lname}_{b}_{h}"))
                            nc.gpsimd.dma_gather(
                                kv_ch[:, toff: toff + nt, :],
                                kv_full[h * cfg.HALF: (h + 1) * cfg.HALF, :],
                                kvidx_s[:, c8: c8 + nt * 8], nt * 128,
                                nt * 128, 256, single_packet=False,
                                queue_num=next_q(), **kw)

                # descriptor pre-generation for the first chunks: desc-gen
                # runs while the kv AllGather is still in flight; the
                # triggers (which carry the kv_full data dep) fire the DMAs
                # the moment the table lands.
                prep_tiles = []
                for ch in chunks[:PREP_AHEAD]:
                    kv_ch = kvpool.tile([128, ch["tiles"], 256], EDT,
                                        tag="kv_ch", name="kv_ch")
                    emit_gathers(ch, kv_ch, True)
                    prep_tiles.append(kv_ch)
                if prep_tiles:
                    for q in range(4):
                        nc.gpsimd.trigger_dma(count=None, queue_num=q)

                for ci, ch in enumerate(chunks):
                    blocks = ch["blocks"]
                    TC = ch["tiles"]
                    tile0 = ch["tile0"]

                    if ci < PREP_AHEAD:
                        kv_ch = prep_tiles[ci]
                    else:
                        kv_ch = kvpool.tile([128, TC, 256], EDT, tag="kv_ch",
                                            name="kv_ch")
                        emit_gathers(ch, kv_ch, False)

                    st_ts, s_ts, q_bs, l_ts, rhs_ts = {}, {}, {}, {}, {}
                    for b in blocks:
                        T = t_lo[b] + t_hi[b]
                        g0 = int(base[b]) * 128
                        st_t = stpool.tile([128, T * 128], cfg.SDT,
                                           tag="ST_b", name="ST_b")
                        nc.sync.dma_start(st_t[:], ST_d[:, g0: g0 + T * 128])
                        st_ts[b] = st_t
                        s_t = sSpool.tile([128, T * 128], cfg.SDT,
                                          tag="S_b", name="S_b")
                        nc.sync.dma_start(s_t[:], S_d[:, g0: g0 + T * 128])
                        s_ts[b] = s_t
                        bsl = slice(b * 128, (b + 1) * 128)
                        q_blk = qpool.tile([128, 128], EDT, tag="q_blk",
                                           name="q_blk")
                        nc.sync.dma_start(q_blk[:], q_tab[bsl, :])
                        q_bs[b] = q_blk

                    # stage 1: qi matmuls + logits
                    for b in blocks:
                        T = t_lo[b] + t_hi[b]
                        toff = int(base[b]) - tile0
                        st_t = st_ts[b]
                        l_t = ltpool.tile([128, T * 4], F32, tag="l_t",
                                          name="l_t")
                        l_ts[b] = l_t
                        for gs in range(0, T, QI):
                            g = min(QI, T - gs)
                            qi_ps = psQ.tile([128, QI, 128], F32, tag="psQ",
                                             name="qi_ps")
                            for i in range(g):
                                c0 = (gs + i) * 128
                                nc.tensor.matmul(
                                    qi_ps[:, i, :],
                                    st_t[:, c0: c0 + 128], q_bs[b][:],
                                    start=True, stop=True)
                            prod = spool.tile([128, QI, 128], EDT,
                                              tag="prod", name="prod")
                            nc.vector.tensor_tensor(
                                prod[:, 0:g, :],
                                qi_ps[:, 0:g, :],
                                kv_ch[:, toff + gs: toff + gs + g, 0:128],
                                mybir.AluOpType.mult)
                            nc.vector.tensor_reduce(
                                l_t[:, gs * 4: (gs + g) * 4].rearrange(
                                    "p (t h) -> p t h", h=4),
                                prod[:, 0:g, :].rearrange(
                                    "p t (h j) -> p t h j", h=4, j=32),
                                mybir.AxisListType.X, mybir.AluOpType.add)

                    # stage 2: exp + v*alpha
                    veng = nc.vector
                    for b in blocks:
                        T = t_lo[b] + t_hi[b]
                        toff = int(base[b]) - tile0
                        rhs = rhspool.tile([128, T, 132], EDT, tag="rhs",
                                           name="rhs")
                        rhs_ts[b] = rhs
                        nc.scalar.activation(
                            rhs[:, :, 0:4],
                            l_ts[b][:].rearrange("p (t h) -> p t h", h=4),
                            EXP, scale=float(cfg.SCALE))
                        veng.tensor_tensor(
                            rhs[:, :, 4:132].rearrange(
                                "p t (h j) -> p t h j", h=4, j=32),
                            kv_ch[:, toff: toff + T, 128:256].rearrange(
                                "p t (h j) -> p t h j", h=4, j=32),
                            rhs[:, :, 0:4].unsqueeze(-1).broadcast_to(
                                [128, T, 4, 32]),
                            mybir.AluOpType.mult)

                    # stage 3: aggregate + normalize + output (+ tab2)
                    for b in blocks:
                        T = t_lo[b] + t_hi[b]
                        bsl = slice(b * 128, (b + 1) * 128)
                        s_t = s_ts[b]
                        rhs = rhs_ts[b]
                        sc_ps = psS.tile([128, 132], F32, tag="psS",
                                         name="sc_ps")
                        for t in range(T):
                            nc.tensor.matmul(
                                sc_ps[:], s_t[:, t * 128: (t + 1) * 128],
                                rhs[:, t, :], start=(t == 0),
                                stop=(t == T - 1))

                        den = spool.tile([128, 4], F32, tag="den", name="den")
                        nc.vector.tensor_scalar_add(den[:], sc_ps[:, 0:4],
                                                    float(cfg.EPS))
                        rec = spool.tile([128, 4], F32, tag="rec", name="rec")
                        nc.vector.reciprocal(rec[:], den[:])
                        h_tmp = spool.tile([128, 128], F32, tag="h_tmp",
                                           name="h_tmp")
                        nc.vector.tensor_tensor(
                            h_tmp[:].rearrange("p (h j) -> p h j", h=4, j=32),
                            sc_ps[:, 4:132].rearrange(
                                "p (h j) -> p h j", h=4, j=32),
                            rec[:].unsqueeze(-1).broadcast_to([128, 4, 32]),
                            mybir.AluOpType.mult)
                        h_blk = spool.tile([128, 128], F32, tag="h_blk",
                                           name="h_blk")
                        nc.scalar.activation(h_blk[:], h_tmp[:], RELU)
                        nc.sync.dma_start(h_rows_out[bsl, :], h_blk[:])
                        if emit_tab2:
                            tr_ps = psT.tile([128, 128], F32, tag="psT",
                                             name="tr_ps")
                            nc.tensor.transpose(tr_ps[:], h_blk[:], id_s[:])
                            h1tb = spool.tile([128, 128], F32, tag="h1tb",
                                              name="h1tb")
                            nc.scalar.activation(h1tb[:], tr_ps[:], COPY)
                            # conv2 q/kv table rows, inline during conv1
                            psq = psS.tile([128, 128], F32, tag="psS",
                                           name="ps_q2")
                            nc.tensor.matmul(psq[:], h1tb[:], w_q2[:],
                                             start=True, stop=False)
                            nc.tensor.matmul(psq[:], ones_s[:], b_q2[:],
                                             start=False, stop=True)
                            qr = spool.tile([128, 128], EDT, tag="qr",
                                            name="qr2")
                            nc.scalar.activation(qr[:], psq[:], COPY)
                            nc.sync.dma_start(q2_tab[bsl, :], qr[:])
                            psk = psT.tile([128, 256], F32, tag="psT",
                                           name="ps_kv2")
                            nc.tensor.matmul(psk[:], h1tb[:], w_kv2[:],
                                             start=True, stop=False)
                            nc.tensor.matmul(psk[:], ones_s[:], b_kv2[:],
                                             start=False, stop=True)
                            kvr = spool.tile([128, 256], EDT, tag="kvr",
                                             name="kvr2")
                            nc.vector.tensor_copy(kvr[:], psk[:])
                            nc.sync.dma_start(kv2_sh[bsl, :], kvr[:])

            emit_conv(q1_tab, kv1_full, h1_rows, True, 'c1')
            nc.gpsimd.collective_compute(
                "AllGather", mybir.AluOpType.bypass, replica_groups=RG,
                ins=[kv2_sh.opt()], outs=[kv2_full.opt()])

            # ================= Q head =================
            def gather_xT(tab, width):
                g = spool.tile([128, 1, width], F32, tag="gx", name="gx")
                nc.gpsimd.dma_gather(g[:], tab[:, :], idxx_s[:],
                                     cfg.B, cfg.B, width)
                tp = psT.tile([128, 128], F32, tag="psT", name="tp_x")
                nc.tensor.transpose(tp[0:width, 0: cfg.B], g[0: cfg.B, 0, :],
                                    id_s[0: cfg.B, 0: cfg.B])
                xt = spool.tile([128, cfg.B], F32, tag="xt", name="xt")
                nc.scalar.activation(xt[0:width, :], tp[0:width, 0: cfg.B],
                                     COPY)
                return xt

            # x1/x2 part (h0/h1 ready once conv1 finished; overlaps conv2)
            x1t = gather_xT(h0_rows, 64)
            x2t = gather_xT(h1_rows, 128)
            zh12_ps = psS.tile([128, cfg.B], F32, tag="psS", name="zh12_ps")
            nc.tensor.matmul(zh12_ps[:], w_qha[:], x1t[0:32, :],
                             start=True, stop=False)
            nc.tensor.matmul(zh12_ps[:], w_qhb[:], x2t[0:128, :],
                             start=False, stop=True)
            zh12 = spool.tile([128, cfg.B], F32, tag="zh12", name="zh12")
            nc.scalar.activation(zh12[:], zh12_ps[:], COPY)

            emit_conv(q2_tab, kv2_full, h2_rows, False, 'c2')

            x3t = gather_xT(h2_rows, 128)
            zh_ps = psQ.tile([128, cfg.B], F32, tag="psQ", name="zh_ps")
            nc.tensor.matmul(zh_ps[:], w_qhc[:], x3t[0:128, :],
                             start=True, stop=True)
            zh = spool.tile([128, cfg.B], F32, tag="zh", name="zh")
            nc.vector.tensor_tensor(zh[:], zh_ps[:], zh12[:],
                                    mybir.AluOpType.add)
            zhr = spool.tile([128, cfg.B], F32, tag="zhr", name="zhr")
            nc.scalar.activation(zhr[:], zh[:], RELU, bias=b_qh[:],
                                 scale=1.0)
            o_ps = psS.tile([cfg.B, 2], F32, tag="psS", name="o_ps")
            nc.tensor.matmul(o_ps[:], zhr[:], w_qh2[:], start=True,
                             stop=False)
            nc.tensor.matmul(o_ps[:], ones_s[:, 0: cfg.B], b_qh2[:],
                             start=False, stop=True)
            ob = spool.tile([cfg.B, 2], F32, tag="ob", name="ob")
            nc.vector.tensor_scalar_mul(ob[:], o_ps[:], mask_s[:])
            nc.sync.dma_start(ar_in[:, :], ob[:])
            nc.gpsimd.collective_compute(
                "AllReduce", mybir.AluOpType.add, replica_groups=RG,
                ins=[ar_in.opt()], outs=[ar_out.opt()])
            nc.sync.dma_start(out_d[:, :], ar_out[:, :])

    nc.compile()
    return nc


# --------------------------------------------------------------------------
# entry point
# --------------------------------------------------------------------------

_trace_flag = {"trace": False}
_last = {}


def _chunk_key(chunks):
    return tuple((tuple(ch["blocks"]), ch["tiles"], ch["tile0"])
                 for ch in chunks)


def _run(inputs, cfg=None):
    cfg = cfg or Cfg()
    in_maps, t_lo, t_hi, chunks, base, slots = _prep_inputs(cfg, inputs)
    key = (slots, tuple(t_lo), tuple(t_hi), _chunk_key(chunks), cfg.edge_bf16)
    if _last.get("key") != key:
        _last["nc"] = build_program(cfg, t_lo, t_hi, chunks, base, slots)
        _last["key"] = key
    nc = _last["nc"]
    res = bass_utils.run_bass_kernel_spmd(
        nc, in_maps, core_ids=list(range(N_CORES)),
        trace=_trace_flag["trace"])
    _last["res"] = res
    return res.results[0]["out"].astype(np.float32)


def kernel(**inputs):
    return _run(inputs)


# revision 23
# speedup vs baseline: 1.5666x; 1.0005x over previous
"""Trainium2 Bass kernel for DGNRNetwork (2-layer TransformerConv GNN + MLPs).

Strategy (8 NeuronCores, graph/data parallel):
  - Nodes padded to N_PAD=50176 and sharded by contiguous range: core c owns
    nodes [c*6272, (c+1)*6272), i.e. 49 blocks of 128 dst nodes per core.
  - Edges partitioned by dst shard on host, laid out BLOCK-MAJOR: for each
    dst block, its lo-half slots then hi-half slots (each padded to whole
    128-edge tiles, uniform across cores -> one SPMD program).  Each block's
    slots are contiguous, so S / S_T / kv_ch slices are contiguous and one
    DMA per block suffices.
  - k||v rows fetched with one indirect DMA (dma_gather) per (block, half),
    round-robined over the 4 SWDGE queues so descriptor generation runs
    concurrently on the 4 GpSimd core pairs.
  - qi = S_T_tile @ Q_blk on TensorE (host-precomputed one-hot S_T).
  - Per-edge logits on Vector (qi straight from PSUM); exp on Scalar into
    rhs[:, :, 0:4]; the v*alpha product reads the exp'd logits broadcast
    (stride-0) so no materialized [128,T,128] attention tile.
  - Segment softmax denominator and weighted sum are ONE accumulated TensorE
    matmul chain with the one-hot scatter matrix S.  Padding edges have
    all-zero S rows so they drop out.
  - Conv loop is stage-batched per chunk (stage1 qi+logits for all blocks,
    stage2 exp+rhs, stage3 aggregate+finish) so the in-order engines overlap
    across blocks instead of ping-ponging inside one block's serial chain.
  - Small weights replicated; kv tables exchanged with AllGather between
    layers; tiny Q-head computed redundantly, combined with masked AllReduce.
"""

import sys

sys.path.insert(0, "/opt/trn_rl_repo")

import numpy as np
import ml_dtypes

import concourse.bacc as bacc
import concourse.bass as bass
import concourse.mybir as mybir
import concourse.tile as tile
from concourse import bass_utils, library_config

F32 = mybir.dt.float32
BF16 = mybir.dt.bfloat16
I16 = mybir.dt.int16

N_CORES = 8


class Cfg:
    def __init__(self, n_nodes=50000, nblk=49, b=64, edge_bf16=True,
                 chunk_tiles=54, qi_group=4):
        self.N = n_nodes
        self.NBLK = nblk                 # dst blocks per core
        self.SHARD = nblk * 128          # nodes per core
        self.N_PAD = 8 * self.SHARD
        self.HALF = self.N_PAD // 2      # kv table split (int16 gather idx)
        self.B = b                       # batch (selected nodes)
        self.F_IN = 128
        self.H = 32
        self.HD = 128
        self.HEADS = 4
        self.EPS = 1e-16
        self.SCALE = 1.0 / np.sqrt(32.0)
        self.edge_bf16 = edge_bf16
        self.EDT = BF16 if edge_bf16 else F32
        self.EDT_NP = ml_dtypes.bfloat16 if edge_bf16 else np.float32
        self.s_fp8 = edge_bf16
        self.SDT = mybir.dt.float8e4 if self.s_fp8 else self.EDT
        self.SDT_NP = ml_dtypes.float8_e4m3 if self.s_fp8 else self.EDT_NP
        self.CHUNK_TILES = chunk_tiles   # max 128-edge tiles per chunk
        self.QI_GROUP = qi_group         # tiles per qi PSUM group (1 bank)
        assert self.N <= self.N_PAD and self.HALF < 32768


# --------------------------------------------------------------------------
# host-side preprocessing
# --------------------------------------------------------------------------


def _wrap16(values, slots):
    """dma_gather idx layout: idx i lives at [i % 16, i // 16], replicated
    across the eight 16-partition groups."""
    arr = np.zeros((16, slots // 16), dtype=np.int16)
    arr[np.arange(len(values)) % 16, np.arange(len(values)) // 16] = values
    return np.tile(arr, (8, 1))


def _plan_chunks(cfg, t_lo, t_hi):
    """Group consecutive blocks into chunks of <= CHUNK_TILES tiles.

    Block-major layout: block b occupies tiles [base[b], base[b]+t_lo[b]+
    t_hi[b]) -- lo tiles then hi tiles, contiguous.  Returns chunks: list of
    dicts with blocks, tile0 (global tile index of chunk start), tiles.
    """
    base = np.zeros(cfg.NBLK + 1, np.int64)
    for b in range(cfg.NBLK):
        base[b + 1] = base[b] + t_lo[b] + t_hi[b]
    chunks = []
    b = 0
    while b < cfg.NBLK:
        blocks = []
        tl = 0
        while b < cfg.NBLK:
            need = t_lo[b] + t_hi[b]
            if blocks and tl + need > cfg.CHUNK_TILES:
                break
            blocks.append(b)
            tl += need
            b += 1
        chunks.append(dict(blocks=blocks, tile0=int(base[blocks[0]]),
                           tiles=tl))
    return chunks, base, int(base[cfg.NBLK])




def _balance_perm(cfg, edge_index):
    """Per-core node->block packing: best-fit-decreasing into blocks capped
    at 1024 in-edges per src half (8 tiles), overflow concentrated in the
    trailing blocks.  Cores keep their node ranges (src lo/hi halves
    invariant); only positions within each shard permute.  Returns
    pos_of_old: old node id -> new node id."""
    esrc = np.asarray(edge_index[0]).astype(np.int64)
    edst = np.asarray(edge_index[1]).astype(np.int64)
    hi = (esrc >= cfg.HALF).astype(np.int64)
    deg = np.zeros((cfg.N_PAD, 2), np.int64)
    np.add.at(deg, (edst, hi), 1)

    CAP = 1024
    K = 3
    pos_of_old = np.zeros(cfg.N_PAD, np.int64)
    nb = cfg.NBLK
    nreg = nb - K
    for c in range(N_CORES):
        ids = np.arange(c * cfg.SHARD, (c + 1) * cfg.SHARD)
        dl = deg[ids, 0]
        dh = deg[ids, 1]
        order = np.argsort(-(dl + dh), kind="stable")
        fl = np.zeros(nb, np.int64)
        fh = np.zeros(nb, np.int64)
        cnt = np.zeros(nb, np.int64)
        assign = np.full(len(ids), -1, np.int64)
        leftover = []
        for i in order:
            ok = ((cnt[:nreg] < 128) & (fl[:nreg] + dl[i] <= CAP)
                  & (fh[:nreg] + dh[i] <= CAP))
            if ok.any():
                cand = np.where(ok)[0]
                score = np.maximum(fl[cand] + dl[i], fh[cand] + dh[i])
                b = int(cand[np.argmax(score)])
            else:
                leftover.append(i)
                continue
            assign[i] = b
            fl[b] += dl[i]
            fh[b] += dh[i]
            cnt[b] += 1
        leftover.sort(key=lambda i: -(dl[i] + dh[i]))
        for i in leftover:
            room = np.where(cnt < 128)[0]
            cand = room[room >= nreg] if (room >= nreg).any() else room
            b = int(cand[np.argmin(np.maximum(fl[cand] + dl[i],
                                              fh[cand] + dh[i]))])
            assign[i] = b
            fl[b] += dl[i]
            fh[b] += dh[i]
            cnt[b] += 1
        binorder = np.lexsort((np.arange(len(ids)), assign))
        rank = np.zeros(len(ids), np.int64)
        pos = np.zeros(nb, np.int64)
        for i in binorder:
            b = assign[i]
            rank[i] = pos[b]
            pos[b] += 1
        pos_of_old[ids] = c * cfg.SHARD + assign * 128 + rank
    return pos_of_old

def _prep_edges(cfg, edge_index):
    src = np.ascontiguousarray(edge_index[0]).astype(np.int64)
    dst = np.ascontiguousarray(edge_index[1]).astype(np.int64)
    core = dst // cfg.SHARD
    blk = (dst % cfg.SHARD) // 128
    hi = (src >= cfg.HALF).astype(np.int64)

    cnt = np.zeros((N_CORES, cfg.NBLK, 2), np.int64)
    np.add.at(cnt, (core, blk, hi), 1)
    t_lo = np.maximum(1, (cnt[:, :, 0].max(0) + 127) // 128)  # [NBLK]
    t_hi = np.maximum(1, (cnt[:, :, 1].max(0) + 127) // 128)

    chunks, base, total_tiles = _plan_chunks(cfg, t_lo.tolist(), t_hi.tolist())
    slots = total_tiles * 128
    assert slots % 16 == 0

    # global slot base for each (blk, hi) group (block-major layout)
    grp_base = np.zeros((cfg.NBLK, 2), np.int64)
    for b in range(cfg.NBLK):
        grp_base[b, 0] = base[b] * 128
        grp_base[b, 1] = (base[b] + t_lo[b]) * 128

    order = np.lexsort((src, hi, blk, core))
    s_src, s_dst, s_core, s_blk, s_hi = (
        src[order], dst[order], core[order], blk[order], hi[order])

    per_core = []
    for c in range(N_CORES):
        m = s_core == c
        csrc, cdst, cblk, chi = s_src[m], s_dst[m], s_blk[m], s_hi[m]
        # composite key non-decreasing under the sort above
        key = cblk * 2 + chi
        kcounts = np.bincount(key, minlength=cfg.NBLK * 2)
        kstarts = np.zeros_like(kcounts)
        kstarts[1:] = np.cumsum(kcounts)[:-1]
        rank = np.arange(len(key)) - kstarts[key]
        slot = grp_base[cblk, chi] + rank

        kv_val = np.where(chi == 1, csrc - cfg.HALF, csrc)
        kv_idx = np.zeros(slots, np.int64)
        kv_idx[slot] = kv_val

        S = np.zeros((128, slots), cfg.SDT_NP)
        scol = (slot // 128) * 128 + (cdst % 128)
        S[slot % 128, scol] = 1.0
        ST = np.zeros((128, slots), cfg.SDT_NP)
        stcol = (slot // 128) * 128 + (slot % 128)
        ST[cdst % 128, stcol] = 1.0

        per_core.append(dict(kv_idx=_wrap16(kv_idx, slots), S=S, ST=ST))
    return per_core, t_lo.tolist(), t_hi.tolist(), chunks, base, slots


def _prep_inputs(cfg, inputs):
    x = np.asarray(inputs["x"], np.float32)
    idx = np.asarray(inputs["idx"]).astype(np.int64)
    f32 = lambda k: np.ascontiguousarray(np.asarray(inputs[k], np.float32))

    ei = np.asarray(inputs["edge_index"]).astype(np.int64)
    pos_of_old = _balance_perm(cfg, ei)
    ei = pos_of_old[ei]
    idx = pos_of_old[idx]

    xp = np.zeros((cfg.N_PAD, cfg.F_IN), np.float32)
    old_of_new = np.argsort(pos_of_old)
    sel = old_of_new[old_of_new < cfg.N]
    xp[pos_of_old[sel]] = x[sel]

    per_core_e, t_lo, t_hi, chunks, base, slots = _prep_edges(cfg, ei)

    wkv1b = np.ascontiguousarray(np.concatenate([
        np.concatenate([f32("c1_wk"), f32("c1_wv")], axis=1),
        np.concatenate([f32("c1_bk"), f32("c1_bv")])[None, :]], axis=0))
    wq1b = np.ascontiguousarray(
        np.concatenate([f32("c1_wq"), f32("c1_bq")[None, :]], axis=0))
    enc_w2b = np.ascontiguousarray(
        np.concatenate([f32("enc_w2"), f32("enc_b2")[None, :]], axis=0))
    wkv2 = np.ascontiguousarray(
        np.concatenate([f32("c2_wk"), f32("c2_wv")], axis=1))     # [128,256]
    bkv2 = np.ascontiguousarray(
        np.concatenate([f32("c2_bk"), f32("c2_bv")])[None, :])
    qw1 = f32("q_w1")                                              # [288,128]
    bpad = ((cfg.B + 127) // 128) * 128

    in_maps = []
    for c in range(N_CORES):
        shard = slice(c * cfg.SHARD, (c + 1) * cfg.SHARD)
        own = (idx // cfg.SHARD) == c
        idx_loc = np.where(own, idx - c * cfg.SHARD, 0)
        im = dict(
            xT=np.ascontiguousarray(
                xp[shard].T.astype(ml_dtypes.bfloat16)),   # [128, SHARD]
            enc_w1=f32("enc_w1").astype(ml_dtypes.bfloat16),
            enc_b1=f32("enc_b1").reshape(32, 1),
            enc_w2=f32("enc_w2").astype(ml_dtypes.bfloat16),
            enc_b2c=f32("enc_b2").reshape(32, 1),
            enc_w2b=enc_w2b.astype(ml_dtypes.bfloat16),
            wq1b=wq1b.astype(ml_dtypes.bfloat16),
            wkv1b=wkv1b.astype(ml_dtypes.bfloat16),
            wq2=f32("c2_wq"), bq2=np.ascontiguousarray(f32("c2_bq")[None, :]),
            wkv2=wkv2, bkv2=bkv2,
            qw1a=np.ascontiguousarray(qw1[0:32]),
            qw1b=np.ascontiguousarray(qw1[32:160]),
            qw1c=np.ascontiguousarray(qw1[160:288]),
            qb1=f32("q_b1").reshape(128, 1),
            qw2=f32("q_w2"),
            qb2=f32("q_b2").reshape(1, 2),
            ones128=np.ones((1, 128), np.float32),
            id128=np.eye(128, dtype=np.float32),
            kv_idx=per_core_e[c]["kv_idx"],
            S_all=per_core_e[c]["S"],
            ST_all=per_core_e[c]["ST"],
            idx_x=_wrap16(idx_loc, bpad),
            own_mask=own.astype(np.float32).reshape(cfg.B, 1),
        )
        in_maps.append(im)
    return in_maps, t_lo, t_hi, chunks, base, slots


# --------------------------------------------------------------------------
# device program
# --------------------------------------------------------------------------


def build_program(cfg, t_lo, t_hi, chunks, base, slots):
    nc = bacc.Bacc("TRN2", target_bir_lowering=False, debug=False,
                   num_devices=N_CORES, num_swdge_queues=4)
    EDT = cfg.EDT
    NB, SH = cfg.NBLK, cfg.SHARD
    RG = [list(range(N_CORES))]
    RELU = mybir.ActivationFunctionType.Relu
    COPY = mybir.ActivationFunctionType.Copy
    EXP = mybir.ActivationFunctionType.Exp

    def din(name, shape, dt=F32):
        return nc.dram_tensor(name, list(shape), dt, kind="ExternalInput").ap()

    xT = din("xT", [128, SH], BF16)
    enc_w1 = din("enc_w1", [128, 32], BF16); enc_b1 = din("enc_b1", [32, 1])
    enc_w2 = din("enc_w2", [32, 32], BF16); enc_b2c = din("enc_b2c", [32, 1])
    enc_w2b = din("enc_w2b", [33, 32], BF16)
    wq1b = din("wq1b", [33, 128], BF16); wkv1b = din("wkv1b", [33, 256], BF16)
    wq2 = din("wq2", [128, 128]); bq2 = din("bq2", [1, 128])
    wkv2 = din("wkv2", [128, 256]); bkv2 = din("bkv2", [1, 256])
    qw1a = din("qw1a", [32, 128]); qw1b = din("qw1b", [128, 128])
    qw1c = din("qw1c", [128, 128]); qb1 = din("qb1", [128, 1])
    qw2 = din("qw2", [128, 2]); qb2 = din("qb2", [1, 2])
    ones128 = din("ones128", [1, 128]); id128 = din("id128", [128, 128])
    kv_idx_d = din("kv_idx", [128, slots // 16], I16)
    S_d = din("S_all", [128, slots], cfg.SDT)
    ST_d = din("ST_all", [128, slots], cfg.SDT)
    bpad = ((cfg.B + 127) // 128) * 128
    idx_x_d = din("idx_x", [128, bpad // 16], I16)
    own_mask_d = din("own_mask", [cfg.B, 1])
    out_d = nc.dram_tensor("out", [cfg.B, 2], F32, kind="ExternalOutput").ap()

    QI = cfg.QI_GROUP

    with tile.TileContext(nc) as tc:
        with (
            tc.tile_pool(name="const", bufs=1) as cpool,
            tc.tile_pool(name="kvch", bufs=3) as kvpool,
            tc.tile_pool(name="stp", bufs=5) as stpool,
            tc.tile_pool(name="sp", bufs=5) as sSpool,
            tc.tile_pool(name="qb", bufs=5) as qpool,
            tc.tile_pool(name="lt", bufs=5) as ltpool,
            tc.tile_pool(name="rhsp", bufs=5) as rhspool,
            tc.tile_pool(name="work", bufs=4) as wpool,
            tc.tile_pool(name="enc", bufs=14) as encpool,
            tc.tile_pool(name="small", bufs=3) as spool,
            tc.tile_pool(name="psQ", bufs=3, space="PSUM") as psQ,
            tc.tile_pool(name="psS", bufs=2, space="PSUM") as psS,
            tc.tile_pool(name="psT", bufs=3, space="PSUM") as psT,
            tc.tile_pool(name="dram", bufs=1, space="DRAM") as dpool,
        ):
            nc.gpsimd.load_library(library_config.mlp)

            def ld(ap, shape, dt=F32, nm=None):
                t = cpool.tile(shape, dt, name=nm or ("ld_" + ap.tensor.name))
                nc.sync.dma_start(t[:], ap[:])
                return t

            w_enc1 = ld(enc_w1, [128, 32], BF16)
            b_enc1 = ld(enc_b1, [32, 1])
            w_enc2 = ld(enc_w2, [32, 32], BF16)
            b_enc2c = ld(enc_b2c, [32, 1])
            w_enc2b = ld(enc_w2b, [33, 32], BF16)
            w_q1b = ld(wq1b, [33, 128], BF16)
            w_kv1b = ld(wkv1b, [33, 256], BF16)
            w_q2 = ld(wq2, [128, 128]); b_q2 = ld(bq2, [1, 128])
            w_kv2 = ld(wkv2, [128, 256]); b_kv2 = ld(bkv2, [1, 256])
            w_qha = ld(qw1a, [32, 128]); w_qhb = ld(qw1b, [128, 128])
            w_qhc = ld(qw1c, [128, 128])
            b_qh = ld(qb1, [128, 1]); w_qh2 = ld(qw2, [128, 2])
            b_qh2 = ld(qb2, [1, 2])
            ones_s = ld(ones128, [1, 128]); id_s = ld(id128, [128, 128])
            nidx16 = (cfg.B + 15) // 16
            idxx_s = cpool.tile([128, nidx16], I16, name="idxx_s")
            nc.sync.dma_start(idxx_s[:], idx_x_d[:, 0:nidx16])
            mask_s = ld(own_mask_d, [cfg.B, 1])
            kvidx_s = cpool.tile([128, slots // 16], I16, name="kvidx_s")
            nc.sync.dma_start(kvidx_s[:], kv_idx_d[:])

            q1_tab = dpool.tile([SH, 128], EDT, name="q1_tab")
            q2_tab = dpool.tile([SH, 128], EDT, name="q2_tab")
            kv1_sh = dpool.tile([SH, 256], EDT, name="kv1_sh")
            kv2_sh = dpool.tile([SH, 256], EDT, name="kv2_sh")
            kv1_full = dpool.tile([cfg.N_PAD, 256], EDT, name="kv1_full",
                                  addr_space="Shared")
            kv2_full = dpool.tile([cfg.N_PAD, 256], EDT, name="kv2_full",
                                  addr_space="Shared")
            h0_rows = dpool.tile([SH, 64], F32, name="h0_rows")
            h1_rows = dpool.tile([SH, 128], F32, name="h1_rows")
            h2_rows = dpool.tile([SH, 128], F32, name="h2_rows")
            ar_in = dpool.tile([cfg.B, 2], F32, name="ar_in")
            ar_out = dpool.tile([cfg.B, 2], F32, name="ar_out",
                                addr_space="Shared")

            # ===== encoder (own shard): kv1 rows first, then AllGather;
            # q1/h0 row emission overlaps the AllGather latency =====
            ENC_G = 4
            kept = []
            for g0 in range(0, NB, ENC_G):
                nb = min(ENC_G, NB - g0)
                W = nb * 128
                gsl = slice(g0 * 128, g0 * 128 + W)
                xch = wpool.tile([128, ENC_G * 128], BF16, tag="xch",
                                 name="xch")
                nc.sync.dma_start(xch[:, 0:W], xT[:, gsl])
                ps1 = psQ.tile([32, ENC_G * 128], F32, tag="psQ",
                               name="ps_enc1")
                nc.tensor.matmul(ps1[:, 0:W], w_enc1[:], xch[:, 0:W],
                                 start=True, stop=True)
                h1p = encpool.tile([33, ENC_G * 128], BF16, tag="h1p",
                                 name="h1p")
                nc.vector.memset(h1p[32:33, :], 1.0)
                nc.scalar.activation(h1p[0:32, 0:W], ps1[:, 0:W], RELU,
                                     bias=b_enc1[:], scale=1.0)
                ps2 = psQ.tile([32, ENC_G * 128], F32, tag="psQ",
                               name="ps_enc2")
                nc.tensor.matmul(ps2[:, 0:W], w_enc2[:], h1p[0:32, 0:W],
                                 start=True, stop=True)
                h0b = encpool.tile([33, ENC_G * 128], BF16, tag="h0b",
                                 name="h0b")
                nc.vector.memset(h0b[32:33, :], 1.0)
                nc.scalar.activation(h0b[0:32, 0:W], ps2[:, 0:W], RELU,
                                     bias=b_enc2c[:], scale=1.0)
                kept.append((g0, nb, h1p, h0b))
                for j in range(nb):
                    b = g0 + j
                    bsl = slice(b * 128, (b + 1) * 128)
                    jsl = slice(j * 128, (j + 1) * 128)
                    psk = psT.tile([128, 256], F32, tag="psT", name="ps_kv")
                    nc.tensor.matmul(psk[:], h0b[:, jsl], w_kv1b[:],
                                     start=True, stop=True)
                    kvr = spool.tile([128, 256], EDT, tag="kvr", name="kvr")
                    nc.vector.tensor_copy(kvr[:], psk[:])
                    nc.sync.dma_start(kv1_sh[bsl, :], kvr[:])
            nc.gpsimd.collective_compute(
                "AllGather", mybir.AluOpType.bypass, replica_groups=RG,
                ins=[kv1_sh.opt()], outs=[kv1_full.opt()])
            # q1 table + h0 rows (overlaps the AllGather)
            for g0, nb, h1p, h0b in kept:
                for j in range(nb):
                    b = g0 + j
                    bsl = slice(b * 128, (b + 1) * 128)
                    jsl = slice(j * 128, (j + 1) * 128)
                    psr = psT.tile([128, 32], F32, tag="psT", name="ps_h0r")
                    nc.tensor.matmul(psr[:], h1p[:, jsl], w_enc2b[:],
                                     start=True, stop=True)
                    h0r = spool.tile([128, 64], F32, tag="h0r", name="h0r")
                    nc.scalar.activation(h0r[:, 0:32], psr[:], RELU)
                    nc.sync.dma_start(h0_rows[bsl, :], h0r[:])
                    psq = psS.tile([128, 128], F32, tag="psS", name="ps_q")
                    nc.tensor.matmul(psq[:], h0b[:, jsl], w_q1b[:],
                                     start=True, stop=True)
                    qr = spool.tile([128, 128], EDT, tag="qr", name="qr")
                    nc.scalar.activation(qr[:], psq[:], COPY)
                    nc.sync.dma_start(q1_tab[bsl, :], qr[:])

            # ================= conv layers =================
            PREP_AHEAD = 0

            def emit_conv(q_tab, kv_full, h_rows_out, emit_tab2, lname):
                qrr = [0]

                def next_q():
                    q = qrr[0] & 3
                    qrr[0] += 1
                    return q

                def emit_gathers(ch, kv_ch, prep):
                    tile0 = ch["tile0"]
                    for b in ch["blocks"]:
                        for h in (0, 1):
                            nt = (t_hi if h else t_lo)[b]
                            toff = int(base[b]) - tile0 + (t_lo[b] if h else 0)
                            c8 = (int(base[b]) + (t_lo[b] if h else 0)) * 8
                            kw = {}
                            if prep:
                                kw = dict(prepare_only=True,
                                          sem=nc.alloc_semaphore(
                                              f"pg_{lname}_{b}_{h}"))
                            nc.gpsimd.dma_gather(
                                kv_ch[:, toff: toff + nt, :],
                                kv_full[h * cfg.HALF: (h + 1) * cfg.HALF, :],
                                kvidx_s[:, c8: c8 + nt * 8], nt * 128,
                                nt * 128, 256, single_packet=False,
                                queue_num=next_q(), **kw)

                # descriptor pre-generation for the first chunks: desc-gen
                # runs while the kv AllGather is still in flight; the
                # triggers (which carry the kv_full data dep) fire the DMAs
                # the moment the table lands.
                prep_tiles = []
                for ch in chunks[:PREP_AHEAD]:
                    kv_ch = kvpool.tile([128, ch["tiles"], 256], EDT,
                                        tag="kv_ch", name="kv_ch")
                    emit_gathers(ch, kv_ch, True)
                    prep_tiles.append(kv_ch)
                if prep_tiles:
                    for q in range(4):
                        nc.gpsimd.trigger_dma(count=None, queue_num=q)

                for ci, ch in enumerate(chunks):
                    blocks = ch["blocks"]
                    TC = ch["tiles"]
                    tile0 = ch["tile0"]

                    if ci < PREP_AHEAD:
                        kv_ch = prep_tiles[ci]
                    else:
                        kv_ch = kvpool.tile([128, TC, 256], EDT, tag="kv_ch",
                                            name="kv_ch")
                        emit_gathers(ch, kv_ch, False)

                    st_ts, s_ts, q_bs, l_ts, rhs_ts = {}, {}, {}, {}, {}
                    for b in blocks:
                        T = t_lo[b] + t_hi[b]
                        g0 = int(base[b]) * 128
                        st_t = stpool.tile([128, T * 128], cfg.SDT,
                                           tag="ST_b", name="ST_b")
                        nc.sync.dma_start(st_t[:], ST_d[:, g0: g0 + T * 128])
                        st_ts[b] = st_t
                        s_t = sSpool.tile([128, T * 128], cfg.SDT,
                                          tag="S_b", name="S_b")
                        nc.sync.dma_start(s_t[:], S_d[:, g0: g0 + T * 128])
                        s_ts[b] = s_t
                        bsl = slice(b * 128, (b + 1) * 128)
                        q_blk = qpool.tile([128, 128], EDT, tag="q_blk",
                                           name="q_blk")
                        nc.sync.dma_start(q_blk[:], q_tab[bsl, :])
                        q_bs[b] = q_blk

                    # stage 1: qi matmuls + logits
                    for b in blocks:
                        T = t_lo[b] + t_hi[b]
                        toff = int(base[b]) - tile0
                        st_t = st_ts[b]
                        l_t = ltpool.tile([128, T * 4], F32, tag="l_t",
                                          name="l_t")
                        l_ts[b] = l_t
                        for gs in range(0, T, QI):
                            g = min(QI, T - gs)
                            qi_ps = psQ.tile([128, QI, 128], F32, tag="psQ",
                                             name="qi_ps")
                            for i in range(g):
                                c0 = (gs + i) * 128
                                nc.tensor.matmul(
                                    qi_ps[:, i, :],
                                    st_t[:, c0: c0 + 128], q_bs[b][:],
                                    start=True, stop=True)
                            prod = spool.tile([128, QI, 128], EDT,
                                              tag="prod", name="prod")
                            nc.vector.tensor_tensor(
                                prod[:, 0:g, :],
                                qi_ps[:, 0:g, :],
                                kv_ch[:, toff + gs: toff + gs + g, 0:128],
                                mybir.AluOpType.mult)
                            nc.vector.tensor_reduce(
                                l_t[:, gs * 4: (gs + g) * 4].rearrange(
                                    "p (t h) -> p t h", h=4),
                                prod[:, 0:g, :].rearrange(
                                    "p t (h j) -> p t h j", h=4, j=32),
                                mybir.AxisListType.X, mybir.AluOpType.add)

                    # stage 2: exp + v*alpha
                    veng = nc.vector
                    for b in blocks:
                        T = t_lo[b] + t_hi[b]
                        toff = int(base[b]) - tile0
                        rhs = rhspool.tile([128, T, 132], EDT, tag="rhs",
                                           name="rhs")
                        rhs_ts[b] = rhs
                        nc.scalar.activation(
                            rhs[:, :, 0:4],
                            l_ts[b][:].rearrange("p (t h) -> p t h", h=4),
                            EXP, scale=float(cfg.SCALE))
                        veng.tensor_tensor(
                            rhs[:, :, 4:132].rearrange(
                                "p t (h j) -> p t h j", h=4, j=32),
                            kv_ch[:, toff: toff + T, 128:256].rearrange(
                                "p t (h j) -> p t h j", h=4, j=32),
                            rhs[:, :, 0:4].unsqueeze(-1).broadcast_to(
                                [128, T, 4, 32]),
                            mybir.AluOpType.mult)

                    # stage 3: aggregate + normalize + output (+ tab2)
                    for b in blocks:
                        T = t_lo[b] + t_hi[b]
                        bsl = slice(b * 128, (b + 1) * 128)
                        s_t = s_ts[b]
                        rhs = rhs_ts[b]
                        sc_ps = psS.tile([128, 132], F32, tag="psS",
                                         name="sc_ps")
                        for t in range(T):
                            nc.tensor.matmul(
                                sc_ps[:], s_t[:, t * 128: (t + 1) * 128],
                                rhs[:, t, :], start=(t == 0),
                                stop=(t == T - 1))

                        den = spool.tile([128, 4], F32, tag="den", name="den")
                        nc.vector.tensor_scalar_add(den[:], sc_ps[:, 0:4],
                                                    float(cfg.EPS))
                        rec = spool.tile([128, 4], F32, tag="rec", name="rec")
                        nc.vector.reciprocal(rec[:], den[:])
                        h_tmp = spool.tile([128, 128], F32, tag="h_tmp",
                                           name="h_tmp")
                        nc.vector.tensor_tensor(
                            h_tmp[:].rearrange("p (h j) -> p h j", h=4, j=32),
                            sc_ps[:, 4:132].rearrange(
                                "p (h j) -> p h j", h=4, j=32),
                            rec[:].unsqueeze(-1).broadcast_to([128, 4, 32]),
                            mybir.AluOpType.mult)
                        h_blk = spool.tile([128, 128], F32, tag="h_blk",
                                           name="h_blk")
                        nc.scalar.activation(h_blk[:], h_tmp[:], RELU)
                        nc.sync.dma_start(h_rows_out[bsl, :], h_blk[:])
                        if emit_tab2:
                            tr_ps = psT.tile([128, 128], F32, tag="psT",
                                             name="tr_ps")
                            nc.tensor.transpose(tr_ps[:], h_blk[:], id_s[:])
                            h1tb = spool.tile([128, 128], F32, tag="h1tb",
                                              name="h1tb")
                            nc.scalar.activation(h1tb[:], tr_ps[:], COPY)
                            # conv2 q/kv table rows, inline during conv1
                            psq = psS.tile([128, 128], F32, tag="psS",
                                           name="ps_q2")
                            nc.tensor.matmul(psq[:], h1tb[:], w_q2[:],
                                             start=True, stop=False)
                            nc.tensor.matmul(psq[:], ones_s[:], b_q2[:],
                                             start=False, stop=True)
                            qr = spool.tile([128, 128], EDT, tag="qr",
                                            name="qr2")
                            nc.scalar.activation(qr[:], psq[:], COPY)
                            nc.sync.dma_start(q2_tab[bsl, :], qr[:])
                            psk = psT.tile([128, 256], F32, tag="psT",
                                           name="ps_kv2")
                            nc.tensor.matmul(psk[:], h1tb[:], w_kv2[:],
                                             start=True, stop=False)
                            nc.tensor.matmul(psk[:], ones_s[:], b_kv2[:],
                                             start=False, stop=True)
                            kvr = spool.tile([128, 256], EDT, tag="kvr",
                                             name="kvr2")
                            nc.vector.tensor_copy(kvr[:], psk[:])
                            nc.sync.dma_start(kv2_sh[bsl, :], kvr[:])

            emit_conv(q1_tab, kv1_full, h1_rows, True, 'c1')
            nc.gpsimd.collective_compute(
                "AllGather", mybir.AluOpType.bypass, replica_groups=RG,
                ins=[kv2_sh.opt()], outs=[kv2_full.opt()])

            # ================= Q head =================
            def gather_xT(tab, width):
                g = spool.tile([128, 1, width], F32, tag="gx", name="gx")
                nc.gpsimd.dma_gather(g[:], tab[:, :], idxx_s[:],
                                     cfg.B, cfg.B, width)
                tp = psT.tile([128, 128], F32, tag="psT", name="tp_x")
                nc.tensor.transpose(tp[0:width, 0: cfg.B], g[0: cfg.B, 0, :],
                                    id_s[0: cfg.B, 0: cfg.B])
                xt = spool.tile([128, cfg.B], F32, tag="xt", name="xt")
                nc.scalar.activation(xt[0:width, :], tp[0:width, 0: cfg.B],
                                     COPY)
                return xt

            # x1/x2 part (h0/h1 ready once conv1 finished; overlaps conv2)
            x1t = gather_xT(h0_rows, 64)
            x2t = gather_xT(h1_rows, 128)
            zh12_ps = psS.tile([128, cfg.B], F32, tag="psS", name="zh12_ps")
            nc.tensor.matmul(zh12_ps[:], w_qha[:], x1t[0:32, :],
                             start=True, stop=False)
            nc.tensor.matmul(zh12_ps[:], w_qhb[:], x2t[0:128, :],
                             start=False, stop=True)
            zh12 = spool.tile([128, cfg.B], F32, tag="zh12", name="zh12")
            nc.scalar.activation(zh12[:], zh12_ps[:], COPY)

            emit_conv(q2_tab, kv2_full, h2_rows, False, 'c2')

            x3t = gather_xT(h2_rows, 128)
            zh_ps = psQ.tile([128, cfg.B], F32, tag="psQ", name="zh_ps")
            nc.tensor.matmul(zh_ps[:], w_qhc[:], x3t[0:128, :],
                             start=True, stop=True)
            zh = spool.tile([128, cfg.B], F32, tag="zh", name="zh")
            nc.vector.tensor_tensor(zh[:], zh_ps[:], zh12[:],
                                    mybir.AluOpType.add)
            zhr = spool.tile([128, cfg.B], F32, tag="zhr", name="zhr")
            nc.scalar.activation(zhr[:], zh[:], RELU, bias=b_qh[:],
                                 scale=1.0)
            o_ps = psS.tile([cfg.B, 2], F32, tag="psS", name="o_ps")
            nc.tensor.matmul(o_ps[:], zhr[:], w_qh2[:], start=True,
                             stop=False)
            nc.tensor.matmul(o_ps[:], ones_s[:, 0: cfg.B], b_qh2[:],
                             start=False, stop=True)
            ob = spool.tile([cfg.B, 2], F32, tag="ob", name="ob")
            nc.vector.tensor_scalar_mul(ob[:], o_ps[:], mask_s[:])
            nc.sync.dma_start(ar_in[:, :], ob[:])
            nc.gpsimd.collective_compute(
                "AllReduce", mybir.AluOpType.add, replica_groups=RG,
                ins=[ar_in.opt()], outs=[ar_out.opt()])
            nc.sync.dma_start(out_d[:, :], ar_out[:, :])

    nc.compile()
    return nc


# --------------------------------------------------------------------------
# entry point
# --------------------------------------------------------------------------

_trace_flag = {"trace": False}
_last = {}


def _chunk_key(chunks):
    return tuple((tuple(ch["blocks"]), ch["tiles"], ch["tile0"])
                 for ch in chunks)


def _run(inputs, cfg=None):
    cfg = cfg or Cfg()
    in_maps, t_lo, t_hi, chunks, base, slots = _prep_inputs(cfg, inputs)
    key = (slots, tuple(t_lo), tuple(t_hi), _chunk_key(chunks), cfg.edge_bf16)
    if _last.get("key") != key:
        _last["nc"] = build_program(cfg, t_lo, t_hi, chunks, base, slots)
        _last["key"] = key
    nc = _last["nc"]
    res = bass_utils.run_bass_kernel_spmd(
        nc, in_maps, core_ids=list(range(N_CORES)),
        trace=_trace_flag["trace"])
    _last["res"] = res
    return res.results[0]["out"].astype(np.float32)


def kernel(**inputs):
    return _run(inputs)
